# revision 24
# baseline (speedup 1.0000x reference)
"""AnchorAttention distributed Bass kernel for 8 TRN2 NeuronCores.

Reference computation (B=2, S=4096, D=1024, H=16, Dh=64, A=512):
  anchors = x[:, :A];  queries = x[:, A:]
  anchor_q/k/v = split_heads(anchors @ Wq/Wk/Wv + b)
  query_q      = split_heads(queries @ Wqt + bqt)
  combined_q   = concat([anchor_q, query_q], axis=2)       # [B,H,S,Dh]
  out  = softmax(combined_q @ anchor_k^T / sqrt(Dh)) @ anchor_v
  out  = merge_heads(out) @ Wo + bo

Sharding: the B*S = 8192 token rows are split into 8 chunks of 1024 rows
(core c -> batch c//4, rows (c%4)*1024 ...). Each core duplicates its
batch's anchor K/V projections, computes Q for its own rows (Wq for the
anchor-region rows, Wqt for query rows), attention over the 512 anchors
for all 16 heads, and the output projection for its rows. The output is a
pure concatenation: no collectives.

Layout: everything is kept transposed ([feature, row]) so each matmul
contracts over the partition dim with zero on-chip transposes; the final
output projection naturally lands un-transposed [row, feature] for DMA
out. Host pre-transposes/pre-casts inputs to bf16 (f32 accumulation in
PSUM). Softmax row-sums come free via an extra all-ones column appended
to V; no max-subtraction is needed (softmax is shift-invariant and the
scores are ~N(0,1)).

Schedule: the PE instruction stream is issue-bound (~0.26us per 512-col
matmul), so the kernel is organized to keep it saturated end to end.
Input DMAs are split per feature-chunk across the two HWDGE dispatch
engines (weights on Sync, anchors/biases on Scalar) so the first K-proj
matmul starts ~2.5us in instead of waiting for whole slabs behind a
serial dispatch queue. Q projection is streamed per head-pair INSIDE the
attention pipeline (group ct+1's tiles are emitted at the tail of stage
ct) so no phase barrier exists anywhere; O-proj partial accumulations
fill the pipeline drain. 1/sums are computed by the DVE fast reciprocal
directly from the PSUM sums rows into fixed parity tiles, cast on the
GPSIMD engine, and partition-broadcast by a single [65,128] 0/1-selector
matmul per row-chunk written into the already-evacuated rows of the
pair's second AV PSUM tile; one mixed-partition-base DVE multiply then
writes the normalized attn^T slab. PSUM evacuations split across the
Scalar/Vector engines; all memsets run on the idle GPSIMD engine.
"""

import numpy as np
import ml_dtypes

import concourse.bass as bass
import concourse.tile as tile
from concourse import bacc, mybir
from concourse import bass_utils

BF16 = mybir.dt.bfloat16
F32 = mybir.dt.float32
B, S, D = 2, 4096, 1024
H, DH = 16, 64
A = 512                  # num_anchor_tokens (asserted at runtime)
RPC = 1024               # rows per core
NCORES = 8
SCALE = 1.0 / np.sqrt(float(DH))

_CACHE = {}


def _build():
    """Build + compile the per-core Bass graph (identical on all cores)."""
    nc = bacc.Bacc("TRN2", target_bir_lowering=False, debug=False)

    xt = nc.dram_tensor("xt", [128, 8, RPC], BF16, kind="ExternalInput")   # rows^T swizzled
    at = nc.dram_tensor("at", [128, 8, A], BF16, kind="ExternalInput")     # anchors^T swizzled
    wlo = nc.dram_tensor("wlo", [128, 8, D], BF16, kind="ExternalInput")   # Q weight rows 0-511
    whi = nc.dram_tensor("whi", [128, 8, D], BF16, kind="ExternalInput")   # Q weight rows 512-1023
    wk = nc.dram_tensor("wk", [128, 8, D], BF16, kind="ExternalInput")
    wv = nc.dram_tensor("wv", [128, 8, D], BF16, kind="ExternalInput")
    wo = nc.dram_tensor("wo", [128, 8, D], BF16, kind="ExternalInput")
    blo = nc.dram_tensor("blo", [128, 8], F32, kind="ExternalInput")
    bhi = nc.dram_tensor("bhi", [128, 8], F32, kind="ExternalInput")
    bk = nc.dram_tensor("bk", [128, 8], F32, kind="ExternalInput")
    bv = nc.dram_tensor("bv", [128, D], F32, kind="ExternalInput")   # pre-broadcast
    bo = nc.dram_tensor("bo", [128, D], F32, kind="ExternalInput")   # pre-broadcast
    out = nc.dram_tensor("out", [RPC, D], F32, kind="ExternalOutput")

    Exp = mybir.ActivationFunctionType.Exp

    from contextlib import ExitStack

    with tile.TileContext(nc) as tc:
        with tc.tile_pool(name="wpool", bufs=1) as wpool, \
             tc.tile_pool(name="cpool", bufs=1) as cpool, \
             tc.tile_pool(name="kvpool", bufs=1) as kvpool, \
             tc.tile_pool(name="qtpool", bufs=2) as qtpool, \
             tc.tile_pool(name="psum", bufs=2, space="PSUM") as psum:
            # wk/wv/at live only through the K/V projections; their pool
            # closes before the attention pools open so the attention
            # working set reuses their SBUF space.
            projstack = ExitStack()
            wearly = projstack.enter_context(
                tc.tile_pool(name="wearly", bufs=1))

            # ---- input DMAs. Big slabs are split per feature-chunk (dt) so
            # subtile deps let consumers start after the first ~400KB; the
            # weight stream dispatches on Sync, anchors+biases on Scalar
            # (the two HWDGE engines dispatch in parallel; each dispatch
            # instruction costs ~0.7us serially on its engine). ----
            def slab_split(pool, t, cols, name, eng, halves=False):
                # whole or half-slab DMAs: keep per-partition lines >= 8KB
                # (fine-grained splits collapse DMA throughput)
                s = pool.tile([128, 8, cols], BF16, name=name)
                if halves:
                    eng.dma_start(out=s[:, 0:4, :], in_=t.ap()[:, 0:4, :])
                    eng.dma_start(out=s[:, 4:8, :], in_=t.ap()[:, 4:8, :])
                else:
                    eng.dma_start(out=s, in_=t.ap())
                return s

            def bias_in(t, name):  # host pre-arranged [128, 8]
                s = cpool.tile([128, 8], F32, name=name)
                nc.scalar.dma_start(out=s, in_=t.ap())
                return s

            def bias_bc(t, name):  # host pre-broadcast [128, D]
                s = cpool.tile([128, D], F32, name=name)
                nc.scalar.dma_start(out=s, in_=t.ap())
                return s

            # K-proj feed first on both queues; then the Q-proj inputs
            # interleaved per feature-chunk (qproj(ct) consumes wlo AND whi
            # from group 0 on) so group 0's scores are never starved.
            wk_sb = slab_split(wearly, wk, D, "wk_sb", nc.sync, halves=True)
            at_sb = wearly.tile([128, 8, A], BF16, name="at_sb")
            nc.gpsimd.dma_start(out=at_sb, in_=at.ap())
            wv_sb = slab_split(wearly, wv, D, "wv_sb", nc.scalar)
            blo_sb = bias_in(blo, "blo_sb")
            bhi_sb = bias_in(bhi, "bhi_sb")
            bk_sb = bias_in(bk, "bk_sb")
            bv_bc = bias_bc(bv, "bv_bc")
            bo_bc = bias_bc(bo, "bo_bc")
            wlo_sb = slab_split(wpool, wlo, D, "wlo_sb", nc.sync)
            xt_sb = slab_split(wpool, xt, RPC, "xt_sb", nc.sync, halves=True)
            whi_sb = slab_split(wpool, whi, D, "whi_sb", nc.sync)
            wo_sb = slab_split(wpool, wo, D, "wo_sb", nc.sync)


            # ---- constants + zero-inits, all on the idle GPSIMD engine ----
            # 0/1 selector for the 1/sums partition broadcast: out rows 0-63
            # take moving row 0, rows 64-127 take moving row 64.
            sel_sb = cpool.tile([65, 128], BF16, name="sel_sb")
            nc.vector.memset(sel_sb, 0.0)
            nc.vector.memset(sel_sb[0:1, 0:64], 1.0)
            nc.vector.memset(sel_sb[64:65, 64:128], 1.0)

            # V slab: [128(a%128), ach, head, 65]; cols 0-63 = V head slice,
            # col 64 = ones (supplies softmax row-sums during AV).
            vaug = kvpool.tile([128, 4, H, DH + 1], BF16, name="vaug")
            nc.vector.memset(vaug, 1.0)

            # fixed parity tiles for the 1/sums chain; rows other than 0/64
            # hold 1.0 forever so the reciprocal/cast stay finite
            sums4s, rcp4s, rcpbfs = [], [], []
            for eo in range(2):
                s4 = cpool.tile([65, 2, 512], F32, name=f"sums4_{eo}")
                nc.vector.memset(s4, 1.0)
                sums4s.append(s4)
                r4 = cpool.tile([65, 2, 512], F32, name=f"rcp4_{eo}")
                rcp4s.append(r4)
                rb = cpool.tile([65, 2, 512], BF16, name=f"rcpbf_{eo}")
                rcpbfs.append(rb)

            qtz = []
            for rc in range(2):
                qt_z0 = qtpool.tile([128, 8, 512], BF16, tag=f"qt0_{rc}",
                                    name=f"qt_z0_{rc}", bufs=1)
                qt_z1 = qtpool.tile([128, 8, 512], BF16, tag=f"qt1_{rc}",
                                    name=f"qt_z1_{rc}", bufs=1)
                nc.vector.memset(qt_z0[64:128, :, :], 0.0)
                nc.vector.memset(qt_z1[0:64, :, :], 0.0)
                qtz.append((qt_z0, qt_z1))

            kt_sb = kvpool.tile([128, 8, A], BF16, name="kt_sb")

            # ---- K^T projection: kt[c, a] = (anchors @ Wk)^T ----
            for ct in range(8):
                pk = psum.tile([128, A], F32, tag="work", name="pk")
                for dt in range(8):
                    nc.tensor.matmul(
                        pk, wk_sb[:, dt, ct * 128:(ct + 1) * 128],
                        at_sb[:, dt, :], start=(dt == 0), stop=(dt == 7))
                nc.vector.tensor_scalar_add(
                    kt_sb[:, ct, :], pk, bk_sb[:, ct:ct + 1])

            # ---- Q^T projection for one head-pair group, written into two
            # zero-padded slabs (z0: odd-head partitions zeroed, z1: even)
            # so score matmuls contract over the full 128 partitions. The pq
            # PSUM tile shares the score tiles' "s" tag/rotation. ----
            def qproj(ct):
                for rc in range(2):
                    wsel = wlo_sb if rc == 0 else whi_sb
                    bsel = blo_sb if rc == 0 else bhi_sb
                    qt_z0, qt_z1 = qtz[rc]
                    pq = psum.tile([128, 512], F32, tag="s", name="pq",
                                   bufs=2)
                    for dt in range(8):
                        nc.tensor.matmul(
                            pq, wsel[:, dt, ct * 128:(ct + 1) * 128],
                            xt_sb[:, dt, rc * 512:(rc + 1) * 512],
                            start=(dt == 0), stop=(dt == 7))
                    nc.vector.tensor_scalar_add(
                        qt_z0[0:64, ct, :], pq[0:64, :], bsel[0:64, ct:ct + 1])
                    nc.vector.tensor_scalar_add(
                        qt_z1[64:128, ct, :], pq[64:128, :],
                        bsel[64:128, ct:ct + 1])

            # ---- V projection (un-transposed): v[a, c] = anchors @ Wv ----
            for ach in range(4):
                for ch in range(2):
                    pv = psum.tile([128, 512], F32, tag="work", name="pv")
                    for dt in range(8):
                        nc.tensor.matmul(
                            pv, at_sb[:, dt, ach * 128:(ach + 1) * 128],
                            wv_sb[:, dt, ch * 512:(ch + 1) * 512],
                            start=(dt == 0), stop=(dt == 7))
                    pv_v = pv.rearrange("p (hd d) -> p hd d", d=DH)
                    bv_v = bv_bc.rearrange(
                        "p (chd hd d) -> p chd hd d", chd=2, d=DH)[:, ch]
                    nc.vector.tensor_add(
                        vaug[:, ach, ch * 8:(ch + 1) * 8, 0:DH], pv_v, bv_v)

            qproj(0)
            qproj(1)

            # ---- attention, software-pipelined over the 8 head-pair
            # groups (ct): scores+exp run one group ahead of AV, two ahead
            # of the normalization; group ct+1's Q projection is emitted at
            # the tail of stage ct so the PE never crosses a phase barrier.
            projstack.close()
            attnstack = ExitStack()
            attnpool = attnstack.enter_context(
                tc.tile_pool(name="attnpool", bufs=1))
            ptpool = attnstack.enter_context(
                tc.tile_pool(name="ptpool", bufs=8))
            tmppool = attnstack.enter_context(
                tc.tile_pool(name="tmppool", bufs=4))
            outpool = attnstack.enter_context(
                tc.tile_pool(name="outpool", bufs=3))
            attnT = attnpool.tile([128, 8, RPC], BF16, name="attnT")

            def stage_scores(ct, mid=None):
                st = {"pts": []}
                for par in range(2):
                    if par == 1 and mid is not None:
                        mid()
                    for rc in range(2):
                        qt_sb = qts_of(rc, par)
                        pt = ptpool.tile([128, 4, 512], BF16, tag="pt",
                                         name="pt")
                        for half in range(2):
                            s2 = psum.tile([128, 2, 512], F32, tag="s",
                                           name="s2", bufs=2)
                            for k in range(2):
                                ach = 2 * half + k
                                nc.tensor.matmul(
                                    s2[:, k, :],
                                    kt_sb[:, ct, ach * 128:(ach + 1) * 128],
                                    qt_sb[:, ct, :],
                                    start=True, stop=True)
                            nc.scalar.activation(
                                out=pt[:, 2 * half:2 * half + 2, :], in_=s2,
                                func=Exp, scale=SCALE)
                        st["pts"].append(pt)
                return st

            def qts_of(rc, par):
                return qtz[rc][par]

            def stage_av(ct, par, st):
                h = 2 * ct + par
                pav = psum.tile([128, 2, 512], F32, tag="work", name="pav",
                                bufs=2)
                for rc in range(2):
                    pt = st["pts"][par * 2 + rc]
                    for ach in range(4):
                        nc.tensor.matmul(
                            pav[0:DH + 1, rc, :], vaug[:, ach, h, :],
                            pt[:, ach, :], start=(ach == 0), stop=(ach == 3))
                if par == 0:
                    st["praw2"] = tmppool.tile([128, 2, 512], BF16,
                                               tag="praw", name="praw2")
                row = par * 64
                # evacuate AV rows split across the Scalar/Vector engines;
                # sums row to the fixed parity SBUF tile (custom-DVE recip
                # cannot read PSUM on hardware, so copy first)
                if par == 0:
                    nc.scalar.copy(st["praw2"][row:row + DH, :, :],
                                   pav[0:DH, :, :])
                else:
                    nc.vector.tensor_copy(st["praw2"][row:row + DH, :, :],
                                          pav[0:DH, :, :])
                nc.vector.tensor_copy(sums4s[ct % 2][row:row + 1, :, :],
                                      pav[DH:DH + 1, :, :])
                st[f"pav{par}"] = pav

            def stage_recip(ct, st):
                nc.vector.reciprocal_approx_fast(rcp4s[ct % 2],
                                                 sums4s[ct % 2])
                nc.vector.tensor_copy(rcpbfs[ct % 2], rcp4s[ct % 2])

            def stage_norm(ct, st):
                pav1 = st["pav1"]
                for rcn in range(2):
                    nc.tensor.matmul(
                        pav1[:, rcn, :], sel_sb, rcpbfs[ct % 2][:, rcn, :],
                        start=True, stop=True)
                dst = attnT[:, ct, :].rearrange("p (b r) -> p b r", b=2)
                nc.vector.tensor_mul(dst, st["praw2"], pav1)

            # O-proj partials for the first four tiles are emitted inside
            # the pipeline drain so the PE has work while the last groups'
            # normalization chains run; the ones at i==8 borrow the then-idle
            # score-tag PSUM slots.
            pouts_head = {}

            def oproj_partial(rti, nh, tag, upto):
                pout = psum.tile([128, 512], F32, tag=tag, name="pout")
                for ct2 in range(upto):
                    nc.tensor.matmul(
                        pout, attnT[:, ct2, rti * 128:(rti + 1) * 128],
                        wo_sb[:, ct2, nh * 512:(nh + 1) * 512],
                        start=(ct2 == 0), stop=False)
                pouts_head[(rti, nh)] = (pout, upto)

            sts = {}
            for i in range(10):
                if i < 8:
                    mid = (lambda ct=i + 2: qproj(ct)) if i < 6 else None
                    sts[i] = stage_scores(i, mid=mid)
                if 1 <= i <= 8:
                    stage_av(i - 1, 0, sts[i - 1])
                if i == 9:
                    oproj_partial(1, 0, "work", 7)
                if 2 <= i <= 9:
                    stage_recip(i - 2, sts[i - 2])
                    stage_norm(i - 2, sts[i - 2])
                if 1 <= i <= 8:
                    stage_av(i - 1, 1, sts[i - 1])
                if i == 8:
                    oproj_partial(0, 0, "s", 7)
                    oproj_partial(0, 1, "s", 7)

            # ---- output projection ----
            oproj_partial(1, 1, "work", 0)
            for rti in range(8):
                for nh in range(2):
                    if (rti, nh) in pouts_head:
                        pout, upto = pouts_head[(rti, nh)]
                        for ct2 in range(upto, 8):
                            nc.tensor.matmul(
                                pout, attnT[:, ct2, rti * 128:(rti + 1) * 128],
                                wo_sb[:, ct2, nh * 512:(nh + 1) * 512],
                                start=(ct2 == 0), stop=(ct2 == 7))
                    else:
                        pout = psum.tile([128, 512], F32, tag="work",
                                         name="pout")
                        for ct2 in range(8):
                            nc.tensor.matmul(
                                pout, attnT[:, ct2, rti * 128:(rti + 1) * 128],
                                wo_sb[:, ct2, nh * 512:(nh + 1) * 512],
                                start=(ct2 == 0), stop=(ct2 == 7))
                    out_t = outpool.tile([128, 512], F32, tag="out",
                                         name="out_t")
                    nc.vector.tensor_add(out_t, pout,
                                         bo_bc[:, nh * 512:(nh + 1) * 512])
                    nc.sync.dma_start(
                        out=out.ap()[rti * 128:(rti + 1) * 128,
                                     nh * 512:(nh + 1) * 512],
                        in_=out_t)
            attnstack.close()

    nc.compile()
    return nc


def _swz(a):
    """[1024, cols] -> [128, 8, cols] with row r -> (r % 128, r // 128)."""
    return np.ascontiguousarray(
        a.reshape(8, 128, -1).transpose(1, 0, 2))


def _make_in_maps(x, Wq, bq, Wk, bk, Wv, bv, Wqt, bqt, Wo, bo):
    x = np.asarray(x, dtype=np.float32)
    bf = ml_dtypes.bfloat16

    wq_b = np.ascontiguousarray(np.asarray(Wq, np.float32).astype(bf))
    wqt_b = np.ascontiguousarray(np.asarray(Wqt, np.float32).astype(bf))
    wk_b = np.ascontiguousarray(np.asarray(Wk, np.float32).astype(bf))
    wv_b = np.ascontiguousarray(np.asarray(Wv, np.float32).astype(bf))
    wo_b = np.ascontiguousarray(np.asarray(Wo, np.float32).astype(bf))
    colmaj = lambda v: np.ascontiguousarray(
        np.asarray(v, np.float32).reshape(8, 128).T)
    bq, bqt, bk = map(colmaj, (bq, bqt, bk))
    bv = np.ascontiguousarray(
        np.broadcast_to(np.asarray(bv, np.float32), (128, D)))
    bo = np.ascontiguousarray(
        np.broadcast_to(np.asarray(bo, np.float32), (128, D)))

    wq_sw, wqt_sw = _swz(wq_b), _swz(wqt_b)
    wk_sw, wv_sw, wo_sw = _swz(wk_b), _swz(wv_b), _swz(wo_b)
    at_sw = [_swz(x[b, :A, :].T.astype(bf)) for b in range(B)]
    in_maps = []
    for c in range(NCORES):
        b, q = divmod(c, 4)
        rows = x[b, q * RPC:(q + 1) * RPC, :]
        in_maps.append({
            "xt": _swz(rows.T.astype(bf)),
            "at": at_sw[b],
            "wlo": wq_sw if q == 0 else wqt_sw,
            "whi": wqt_sw,
            "wk": wk_sw, "wv": wv_sw, "wo": wo_sw,
            "blo": bq if q == 0 else bqt, "bhi": bqt,
            "bk": bk, "bv": bv, "bo": bo,
        })
    return in_maps


def kernel(x, Wq, bq, Wk, bk, Wv, bv, Wqt, bqt, Wo, bo, num_anchor_tokens):
    assert int(num_anchor_tokens) == A
    if "nc" not in _CACHE:
        _CACHE["nc"] = _build()
    nc = _CACHE["nc"]

    in_maps = _make_in_maps(x, Wq, bq, Wk, bk, Wv, bv, Wqt, bqt, Wo, bo)
    res = bass_utils.run_bass_kernel_spmd(
        nc, in_maps, core_ids=list(range(NCORES)))
    out = np.empty((B, S, D), np.float32)
    for c in range(NCORES):
        b, q = divmod(c, 4)
        out[b, q * RPC:(q + 1) * RPC, :] = res.results[c]["out"]
    return out


# revision 25
# speedup vs baseline: 1.0321x; 1.0321x over previous
"""AnchorAttention distributed Bass kernel for 8 TRN2 NeuronCores.

Reference computation (B=2, S=4096, D=1024, H=16, Dh=64, A=512):
  anchors = x[:, :A];  queries = x[:, A:]
  anchor_q/k/v = split_heads(anchors @ Wq/Wk/Wv + b)
  query_q      = split_heads(queries @ Wqt + bqt)
  combined_q   = concat([anchor_q, query_q], axis=2)       # [B,H,S,Dh]
  out  = softmax(combined_q @ anchor_k^T / sqrt(Dh)) @ anchor_v
  out  = merge_heads(out) @ Wo + bo

Sharding: the B*S = 8192 token rows are split into 8 chunks of 1024 rows
(core c -> batch c//4, rows (c%4)*1024 ...). Each core duplicates its
batch's anchor K/V projections, computes Q for its own rows (Wq for the
anchor-region rows, Wqt for query rows), attention over the 512 anchors
for all 16 heads, and the output projection for its rows. The output is a
pure concatenation: no collectives.

Layout: everything is kept transposed ([feature, row]) so each matmul
contracts over the partition dim with zero on-chip transposes; the final
output projection naturally lands un-transposed [row, feature] for DMA
out. Host pre-transposes/pre-casts inputs to bf16 (f32 accumulation in
PSUM). Softmax row-sums come free via an extra all-ones column appended
to V; no max-subtraction is needed (softmax is shift-invariant and the
scores are ~N(0,1)).

Schedule: the PE instruction stream is issue-bound (~0.26us per 512-col
matmul), so the kernel is organized to keep it saturated end to end.
Input DMAs are split per feature-chunk across the two HWDGE dispatch
engines (weights on Sync, anchors/biases on Scalar) so the first K-proj
matmul starts ~2.5us in instead of waiting for whole slabs behind a
serial dispatch queue. Q projection is streamed per head-pair INSIDE the
attention pipeline (group ct+1's tiles are emitted at the tail of stage
ct) so no phase barrier exists anywhere; O-proj partial accumulations
fill the pipeline drain. 1/sums are computed by the DVE fast reciprocal
directly from the PSUM sums rows into fixed parity tiles, cast on the
GPSIMD engine, and partition-broadcast by a single [65,128] 0/1-selector
matmul per row-chunk written into the already-evacuated rows of the
pair's second AV PSUM tile; one mixed-partition-base DVE multiply then
writes the normalized attn^T slab. PSUM evacuations split across the
Scalar/Vector engines; all memsets run on the idle GPSIMD engine.
"""

import numpy as np
import ml_dtypes

import concourse.bass as bass
import concourse.tile as tile
from concourse import bacc, mybir
from concourse import bass_utils

BF16 = mybir.dt.bfloat16
F32 = mybir.dt.float32
B, S, D = 2, 4096, 1024
H, DH = 16, 64
A = 512                  # num_anchor_tokens (asserted at runtime)
RPC = 1024               # rows per core
NCORES = 8
SCALE = 1.0 / np.sqrt(float(DH))

_CACHE = {}


def _build():
    """Build + compile the per-core Bass graph (identical on all cores)."""
    nc = bacc.Bacc("TRN2", target_bir_lowering=False, debug=False)

    xt = nc.dram_tensor("xt", [128, 8, RPC], BF16, kind="ExternalInput")   # rows^T swizzled
    at = nc.dram_tensor("at", [128, 8, A], BF16, kind="ExternalInput")     # anchors^T swizzled
    wlo = nc.dram_tensor("wlo", [128, 8, D], BF16, kind="ExternalInput")   # Q weight rows 0-511
    whi = nc.dram_tensor("whi", [128, 8, D], BF16, kind="ExternalInput")   # Q weight rows 512-1023
    wk = nc.dram_tensor("wk", [128, 8, D], BF16, kind="ExternalInput")
    wv = nc.dram_tensor("wv", [128, 8, D], BF16, kind="ExternalInput")
    wo = nc.dram_tensor("wo", [128, 8, D], BF16, kind="ExternalInput")
    blo = nc.dram_tensor("blo", [128, 8], F32, kind="ExternalInput")
    bhi = nc.dram_tensor("bhi", [128, 8], F32, kind="ExternalInput")
    bk = nc.dram_tensor("bk", [128, 8], F32, kind="ExternalInput")
    bv = nc.dram_tensor("bv", [128, D], F32, kind="ExternalInput")   # pre-broadcast
    bo = nc.dram_tensor("bo", [128, D], F32, kind="ExternalInput")   # pre-broadcast
    out = nc.dram_tensor("out", [RPC, D], F32, kind="ExternalOutput")

    Exp = mybir.ActivationFunctionType.Exp

    from contextlib import ExitStack

    with tile.TileContext(nc) as tc:
        with tc.tile_pool(name="wpool", bufs=1) as wpool, \
             tc.tile_pool(name="cpool", bufs=1) as cpool, \
             tc.tile_pool(name="kvpool", bufs=1) as kvpool, \
             tc.tile_pool(name="qtpool", bufs=2) as qtpool, \
             tc.tile_pool(name="psum", bufs=2, space="PSUM") as psum:
            # wk/wv/at live only through the K/V projections; their pool
            # closes before the attention pools open so the attention
            # working set reuses their SBUF space.
            projstack = ExitStack()
            wearly = projstack.enter_context(
                tc.tile_pool(name="wearly", bufs=1))

            # ---- input DMAs. Big slabs are split per feature-chunk (dt) so
            # subtile deps let consumers start after the first ~400KB; the
            # weight stream dispatches on Sync, anchors+biases on Scalar
            # (the two HWDGE engines dispatch in parallel; each dispatch
            # instruction costs ~0.7us serially on its engine). ----
            def slab_split(pool, t, cols, name, eng, halves=False):
                # whole or half-slab DMAs: keep per-partition lines >= 8KB
                # (fine-grained splits collapse DMA throughput)
                s = pool.tile([128, 8, cols], BF16, name=name)
                if halves:
                    eng.dma_start(out=s[:, 0:4, :], in_=t.ap()[:, 0:4, :])
                    eng.dma_start(out=s[:, 4:8, :], in_=t.ap()[:, 4:8, :])
                else:
                    eng.dma_start(out=s, in_=t.ap())
                return s

            def bias_in(t, name):  # host pre-arranged [128, 8]
                s = cpool.tile([128, 8], F32, name=name)
                nc.scalar.dma_start(out=s, in_=t.ap())
                return s

            def bias_bc(t, name):  # host pre-broadcast [128, D]
                s = cpool.tile([128, D], F32, name=name)
                nc.scalar.dma_start(out=s, in_=t.ap())
                return s

            # K-proj feed first on both queues; then the Q-proj inputs
            # interleaved per feature-chunk (qproj(ct) consumes wlo AND whi
            # from group 0 on) so group 0's scores are never starved.
            wk_sb = slab_split(wearly, wk, D, "wk_sb", nc.sync)
            at_sb = wearly.tile([128, 8, A], BF16, name="at_sb")
            nc.scalar.dma_start(out=at_sb, in_=at.ap())
            wv_sb = slab_split(wearly, wv, D, "wv_sb", nc.sync)
            blo_sb = bias_in(blo, "blo_sb")
            bhi_sb = bias_in(bhi, "bhi_sb")
            bk_sb = bias_in(bk, "bk_sb")
            bv_bc = bias_bc(bv, "bv_bc")
            bo_bc = bias_bc(bo, "bo_bc")
            whi_sb = slab_split(wpool, whi, D, "whi_sb", nc.scalar)
            wlo_sb = slab_split(wpool, wlo, D, "wlo_sb", nc.sync)
            xt_sb = slab_split(wpool, xt, RPC, "xt_sb", nc.sync)
            wo_sb = slab_split(wpool, wo, D, "wo_sb", nc.sync)


            # ---- constants + zero-inits, all on the idle GPSIMD engine ----
            # 0/1 selector for the 1/sums partition broadcast: out rows 0-63
            # take moving row 0, rows 64-127 take moving row 64.
            sel_sb = cpool.tile([65, 128], BF16, name="sel_sb")
            nc.vector.memset(sel_sb, 0.0)
            nc.vector.memset(sel_sb[0:1, 0:64], 1.0)
            nc.vector.memset(sel_sb[64:65, 64:128], 1.0)

            # V slab: [128(a%128), ach, head, 65]; cols 0-63 = V head slice,
            # col 64 = ones (supplies softmax row-sums during AV).
            vaug = kvpool.tile([128, 4, H, DH + 1], BF16, name="vaug")
            nc.vector.memset(vaug, 1.0)

            # fixed parity tiles for the 1/sums chain; rows other than 0/64
            # hold 1.0 forever so the reciprocal/cast stay finite
            sums4s, rcp4s, rcpbfs = [], [], []
            for eo in range(2):
                s4 = cpool.tile([65, 2, 512], F32, name=f"sums4_{eo}")
                nc.vector.memset(s4, 1.0)
                sums4s.append(s4)
                r4 = cpool.tile([65, 2, 512], F32, name=f"rcp4_{eo}")
                rcp4s.append(r4)
                rb = cpool.tile([65, 2, 512], BF16, name=f"rcpbf_{eo}")
                rcpbfs.append(rb)

            qtz = []
            for rc in range(2):
                qt_z0 = qtpool.tile([128, 8, 512], BF16, tag=f"qt0_{rc}",
                                    name=f"qt_z0_{rc}", bufs=1)
                qt_z1 = qtpool.tile([128, 8, 512], BF16, tag=f"qt1_{rc}",
                                    name=f"qt_z1_{rc}", bufs=1)
                nc.vector.memset(qt_z0[64:128, :, :], 0.0)
                nc.vector.memset(qt_z1[0:64, :, :], 0.0)
                qtz.append((qt_z0, qt_z1))

            kt_sb = kvpool.tile([128, 8, A], BF16, name="kt_sb")

            # ---- K^T projection: kt[c, a] = (anchors @ Wk)^T ----
            for ct in range(8):
                pk = psum.tile([128, A], F32, tag="work", name="pk")
                for dt in range(8):
                    nc.tensor.matmul(
                        pk, wk_sb[:, dt, ct * 128:(ct + 1) * 128],
                        at_sb[:, dt, :], start=(dt == 0), stop=(dt == 7))
                nc.vector.tensor_scalar_add(
                    kt_sb[:, ct, :], pk, bk_sb[:, ct:ct + 1])

            # ---- Q^T projection for one head-pair group, written into two
            # zero-padded slabs (z0: odd-head partitions zeroed, z1: even)
            # so score matmuls contract over the full 128 partitions. The pq
            # PSUM tile shares the score tiles' "s" tag/rotation. ----
            def qproj(ct):
                for rc in range(2):
                    wsel = wlo_sb if rc == 0 else whi_sb
                    bsel = blo_sb if rc == 0 else bhi_sb
                    qt_z0, qt_z1 = qtz[rc]
                    pq = psum.tile([128, 512], F32, tag="s", name="pq",
                                   bufs=2)
                    for dt in range(8):
                        nc.tensor.matmul(
                            pq, wsel[:, dt, ct * 128:(ct + 1) * 128],
                            xt_sb[:, dt, rc * 512:(rc + 1) * 512],
                            start=(dt == 0), stop=(dt == 7))
                    nc.vector.tensor_scalar_add(
                        qt_z0[0:64, ct, :], pq[0:64, :], bsel[0:64, ct:ct + 1])
                    nc.vector.tensor_scalar_add(
                        qt_z1[64:128, ct, :], pq[64:128, :],
                        bsel[64:128, ct:ct + 1])

            # ---- V projection (un-transposed): v[a, c] = anchors @ Wv ----
            for ach in range(4):
                for ch in range(2):
                    pv = psum.tile([128, 512], F32, tag="work", name="pv")
                    for dt in range(8):
                        nc.tensor.matmul(
                            pv, at_sb[:, dt, ach * 128:(ach + 1) * 128],
                            wv_sb[:, dt, ch * 512:(ch + 1) * 512],
                            start=(dt == 0), stop=(dt == 7))
                    pv_v = pv.rearrange("p (hd d) -> p hd d", d=DH)
                    bv_v = bv_bc.rearrange(
                        "p (chd hd d) -> p chd hd d", chd=2, d=DH)[:, ch]
                    nc.vector.tensor_add(
                        vaug[:, ach, ch * 8:(ch + 1) * 8, 0:DH], pv_v, bv_v)

            qproj(0)
            qproj(1)

            # ---- attention, software-pipelined over the 8 head-pair
            # groups (ct): scores+exp run one group ahead of AV, two ahead
            # of the normalization; group ct+1's Q projection is emitted at
            # the tail of stage ct so the PE never crosses a phase barrier.
            projstack.close()
            attnstack = ExitStack()
            attnpool = attnstack.enter_context(
                tc.tile_pool(name="attnpool", bufs=1))
            ptpool = attnstack.enter_context(
                tc.tile_pool(name="ptpool", bufs=8))
            tmppool = attnstack.enter_context(
                tc.tile_pool(name="tmppool", bufs=4))
            outpool = attnstack.enter_context(
                tc.tile_pool(name="outpool", bufs=3))
            attnT = attnpool.tile([128, 8, RPC], BF16, name="attnT")

            def stage_scores(ct, mid=None):
                st = {"pts": []}
                for par in range(2):
                    if par == 1 and mid is not None:
                        mid()
                    for rc in range(2):
                        qt_sb = qts_of(rc, par)
                        pt = ptpool.tile([128, 4, 512], BF16, tag="pt",
                                         name="pt")
                        for half in range(2):
                            s2 = psum.tile([128, 2, 512], F32, tag="s",
                                           name="s2", bufs=2)
                            for k in range(2):
                                ach = 2 * half + k
                                nc.tensor.matmul(
                                    s2[:, k, :],
                                    kt_sb[:, ct, ach * 128:(ach + 1) * 128],
                                    qt_sb[:, ct, :],
                                    start=True, stop=True)
                            nc.scalar.activation(
                                out=pt[:, 2 * half:2 * half + 2, :], in_=s2,
                                func=Exp, scale=SCALE)
                        st["pts"].append(pt)
                return st

            def qts_of(rc, par):
                return qtz[rc][par]

            def stage_av(ct, par, st):
                h = 2 * ct + par
                pav = psum.tile([128, 2, 512], F32, tag="work", name="pav",
                                bufs=2)
                for rc in range(2):
                    pt = st["pts"][par * 2 + rc]
                    for ach in range(4):
                        nc.tensor.matmul(
                            pav[0:DH + 1, rc, :], vaug[:, ach, h, :],
                            pt[:, ach, :], start=(ach == 0), stop=(ach == 3))
                if par == 0:
                    st["praw2"] = tmppool.tile([128, 2, 512], BF16,
                                               tag="praw", name="praw2")
                row = par * 64
                # evacuate AV rows split across the Scalar/Vector engines;
                # sums row to the fixed parity SBUF tile (custom-DVE recip
                # cannot read PSUM on hardware, so copy first)
                if par == 0:
                    nc.scalar.copy(st["praw2"][row:row + DH, :, :],
                                   pav[0:DH, :, :])
                else:
                    nc.vector.tensor_copy(st["praw2"][row:row + DH, :, :],
                                          pav[0:DH, :, :])
                nc.vector.tensor_copy(sums4s[ct % 2][row:row + 1, :, :],
                                      pav[DH:DH + 1, :, :])
                st[f"pav{par}"] = pav

            def stage_recip(ct, st):
                nc.vector.reciprocal_approx_fast(rcp4s[ct % 2],
                                                 sums4s[ct % 2])
                nc.vector.tensor_copy(rcpbfs[ct % 2], rcp4s[ct % 2])

            def stage_norm(ct, st):
                pav1 = st["pav1"]
                for rcn in range(2):
                    nc.tensor.matmul(
                        pav1[:, rcn, :], sel_sb, rcpbfs[ct % 2][:, rcn, :],
                        start=True, stop=True)
                dst = attnT[:, ct, :].rearrange("p (b r) -> p b r", b=2)
                nc.vector.tensor_mul(dst, st["praw2"], pav1)

            # O-proj partials for the first four tiles are emitted inside
            # the pipeline drain so the PE has work while the last groups'
            # normalization chains run; the ones at i==8 borrow the then-idle
            # score-tag PSUM slots.
            pouts_head = {}

            def oproj_partial(rti, nh, tag, upto):
                pout = psum.tile([128, 512], F32, tag=tag, name="pout")
                for ct2 in range(upto):
                    nc.tensor.matmul(
                        pout, attnT[:, ct2, rti * 128:(rti + 1) * 128],
                        wo_sb[:, ct2, nh * 512:(nh + 1) * 512],
                        start=(ct2 == 0), stop=False)
                pouts_head[(rti, nh)] = (pout, upto)

            sts = {}
            for i in range(10):
                if i < 8:
                    mid = (lambda ct=i + 2: qproj(ct)) if i < 6 else None
                    sts[i] = stage_scores(i, mid=mid)
                if 1 <= i <= 8:
                    stage_av(i - 1, 0, sts[i - 1])
                if i == 9:
                    oproj_partial(1, 0, "work", 7)
                if 2 <= i <= 9:
                    stage_recip(i - 2, sts[i - 2])
                    stage_norm(i - 2, sts[i - 2])
                if 1 <= i <= 8:
                    stage_av(i - 1, 1, sts[i - 1])
                if i == 8:
                    oproj_partial(0, 0, "s", 7)
                    oproj_partial(0, 1, "s", 7)

            # ---- output projection ----
            oproj_partial(1, 1, "work", 0)
            for rti in range(8):
                for nh in range(2):
                    if (rti, nh) in pouts_head:
                        pout, upto = pouts_head[(rti, nh)]
                        for ct2 in range(upto, 8):
                            nc.tensor.matmul(
                                pout, attnT[:, ct2, rti * 128:(rti + 1) * 128],
                                wo_sb[:, ct2, nh * 512:(nh + 1) * 512],
                                start=(ct2 == 0), stop=(ct2 == 7))
                    else:
                        pout = psum.tile([128, 512], F32, tag="work",
                                         name="pout")
                        for ct2 in range(8):
                            nc.tensor.matmul(
                                pout, attnT[:, ct2, rti * 128:(rti + 1) * 128],
                                wo_sb[:, ct2, nh * 512:(nh + 1) * 512],
                                start=(ct2 == 0), stop=(ct2 == 7))
                    out_t = outpool.tile([128, 512], F32, tag="out",
                                         name="out_t")
                    nc.vector.tensor_add(out_t, pout,
                                         bo_bc[:, nh * 512:(nh + 1) * 512])
                    nc.sync.dma_start(
                        out=out.ap()[rti * 128:(rti + 1) * 128,
                                     nh * 512:(nh + 1) * 512],
                        in_=out_t)
            attnstack.close()

    nc.compile()
    return nc


def _swz(a):
    """[1024, cols] -> [128, 8, cols] with row r -> (r % 128, r // 128)."""
    return np.ascontiguousarray(
        a.reshape(8, 128, -1).transpose(1, 0, 2))


def _make_in_maps(x, Wq, bq, Wk, bk, Wv, bv, Wqt, bqt, Wo, bo):
    x = np.asarray(x, dtype=np.float32)
    bf = ml_dtypes.bfloat16

    wq_b = np.ascontiguousarray(np.asarray(Wq, np.float32).astype(bf))
    wqt_b = np.ascontiguousarray(np.asarray(Wqt, np.float32).astype(bf))
    wk_b = np.ascontiguousarray(np.asarray(Wk, np.float32).astype(bf))
    wv_b = np.ascontiguousarray(np.asarray(Wv, np.float32).astype(bf))
    wo_b = np.ascontiguousarray(np.asarray(Wo, np.float32).astype(bf))
    colmaj = lambda v: np.ascontiguousarray(
        np.asarray(v, np.float32).reshape(8, 128).T)
    bq, bqt, bk = map(colmaj, (bq, bqt, bk))
    bv = np.ascontiguousarray(
        np.broadcast_to(np.asarray(bv, np.float32), (128, D)))
    bo = np.ascontiguousarray(
        np.broadcast_to(np.asarray(bo, np.float32), (128, D)))

    wq_sw, wqt_sw = _swz(wq_b), _swz(wqt_b)
    wk_sw, wv_sw, wo_sw = _swz(wk_b), _swz(wv_b), _swz(wo_b)
    at_sw = [_swz(x[b, :A, :].T.astype(bf)) for b in range(B)]
    in_maps = []
    for c in range(NCORES):
        b, q = divmod(c, 4)
        rows = x[b, q * RPC:(q + 1) * RPC, :]
        in_maps.append({
            "xt": _swz(rows.T.astype(bf)),
            "at": at_sw[b],
            "wlo": wq_sw if q == 0 else wqt_sw,
            "whi": wqt_sw,
            "wk": wk_sw, "wv": wv_sw, "wo": wo_sw,
            "blo": bq if q == 0 else bqt, "bhi": bqt,
            "bk": bk, "bv": bv, "bo": bo,
        })
    return in_maps


def kernel(x, Wq, bq, Wk, bk, Wv, bv, Wqt, bqt, Wo, bo, num_anchor_tokens):
    assert int(num_anchor_tokens) == A
    if "nc" not in _CACHE:
        _CACHE["nc"] = _build()
    nc = _CACHE["nc"]

    in_maps = _make_in_maps(x, Wq, bq, Wk, bk, Wv, bv, Wqt, bqt, Wo, bo)
    res = bass_utils.run_bass_kernel_spmd(
        nc, in_maps, core_ids=list(range(NCORES)))
    out = np.empty((B, S, D), np.float32)
    for c in range(NCORES):
        b, q = divmod(c, 4)
        out[b, q * RPC:(q + 1) * RPC, :] = res.results[c]["out"]
    return out


# revision 27
# speedup vs baseline: 1.0476x; 1.0150x over previous
"""AnchorAttention distributed Bass kernel for 8 TRN2 NeuronCores.

Reference computation (B=2, S=4096, D=1024, H=16, Dh=64, A=512):
  anchors = x[:, :A];  queries = x[:, A:]
  anchor_q/k/v = split_heads(anchors @ Wq/Wk/Wv + b)
  query_q      = split_heads(queries @ Wqt + bqt)
  combined_q   = concat([anchor_q, query_q], axis=2)       # [B,H,S,Dh]
  out  = softmax(combined_q @ anchor_k^T / sqrt(Dh)) @ anchor_v
  out  = merge_heads(out) @ Wo + bo

Sharding: the B*S = 8192 token rows are split into 8 chunks of 1024 rows
(core c -> batch c//4, rows (c%4)*1024 ...). Each core duplicates its
batch's anchor K/V projections, computes Q for its own rows (Wq for the
anchor-region rows, Wqt for query rows), attention over the 512 anchors
for all 16 heads, and the output projection for its rows. The output is a
pure concatenation: no collectives.

Layout: everything is kept transposed ([feature, row]) so each matmul
contracts over the partition dim with zero on-chip transposes; the final
output projection naturally lands un-transposed [row, feature] for DMA
out. Host pre-transposes/pre-casts inputs to bf16 (f32 accumulation in
PSUM). Softmax row-sums come free via an extra all-ones column appended
to V; no max-subtraction is needed (softmax is shift-invariant and the
scores are ~N(0,1)).

Schedule: the PE instruction stream is issue-bound (~0.26us per 512-col
matmul), so the kernel is organized to keep it saturated end to end.
Input DMAs are split per feature-chunk across the two HWDGE dispatch
engines (weights on Sync, anchors/biases on Scalar) so the first K-proj
matmul starts ~2.5us in instead of waiting for whole slabs behind a
serial dispatch queue. Q projection is streamed per head-pair INSIDE the
attention pipeline (group ct+1's tiles are emitted at the tail of stage
ct) so no phase barrier exists anywhere; O-proj partial accumulations
fill the pipeline drain. 1/sums are computed by the DVE fast reciprocal
directly from the PSUM sums rows into fixed parity tiles, cast on the
GPSIMD engine, and partition-broadcast by a single [65,128] 0/1-selector
matmul per row-chunk written into the already-evacuated rows of the
pair's second AV PSUM tile; one mixed-partition-base DVE multiply then
writes the normalized attn^T slab. PSUM evacuations split across the
Scalar/Vector engines; all memsets run on the idle GPSIMD engine.
"""

import numpy as np
import ml_dtypes

import concourse.bass as bass
import concourse.tile as tile
from concourse import bacc, mybir
from concourse import bass_utils

BF16 = mybir.dt.bfloat16
F32 = mybir.dt.float32
B, S, D = 2, 4096, 1024
H, DH = 16, 64
A = 512                  # num_anchor_tokens (asserted at runtime)
RPC = 1024               # rows per core
NCORES = 8
SCALE = 1.0 / np.sqrt(float(DH))

_CACHE = {}


def _build():
    """Build + compile the per-core Bass graph (identical on all cores)."""
    nc = bacc.Bacc("TRN2", target_bir_lowering=False, debug=False)

    xt = nc.dram_tensor("xt", [128, 8, RPC], BF16, kind="ExternalInput")   # rows^T swizzled
    at = nc.dram_tensor("at", [128, 8, A], BF16, kind="ExternalInput")     # anchors^T swizzled
    wlo = nc.dram_tensor("wlo", [128, 8, D], BF16, kind="ExternalInput")   # Q weight rows 0-511
    whi = nc.dram_tensor("whi", [128, 8, D], BF16, kind="ExternalInput")   # Q weight rows 512-1023
    wk = nc.dram_tensor("wk", [128, 8, D], BF16, kind="ExternalInput")
    wv = nc.dram_tensor("wv", [128, 8, D], BF16, kind="ExternalInput")
    wo = nc.dram_tensor("wo", [128, 8, D], BF16, kind="ExternalInput")
    blo = nc.dram_tensor("blo", [128, 8], F32, kind="ExternalInput")
    bhi = nc.dram_tensor("bhi", [128, 8], F32, kind="ExternalInput")
    bk = nc.dram_tensor("bk", [128, 8], F32, kind="ExternalInput")
    bv = nc.dram_tensor("bv", [128, D], F32, kind="ExternalInput")   # pre-broadcast
    bo = nc.dram_tensor("bo", [128, D], F32, kind="ExternalInput")   # pre-broadcast
    out = nc.dram_tensor("out", [RPC, D], F32, kind="ExternalOutput")

    Exp = mybir.ActivationFunctionType.Exp

    from contextlib import ExitStack

    with tile.TileContext(nc) as tc:
        with tc.tile_pool(name="wpool", bufs=1) as wpool, \
             tc.tile_pool(name="cpool", bufs=1) as cpool, \
             tc.tile_pool(name="kvpool", bufs=1) as kvpool, \
             tc.tile_pool(name="qtpool", bufs=2) as qtpool, \
             tc.tile_pool(name="psum", bufs=2, space="PSUM") as psum:
            # wk/wv/at live only through the K/V projections; their pool
            # closes before the attention pools open so the attention
            # working set reuses their SBUF space.
            projstack = ExitStack()
            wearly = projstack.enter_context(
                tc.tile_pool(name="wearly", bufs=1))

            # ---- input DMAs. Big slabs are split per feature-chunk (dt) so
            # subtile deps let consumers start after the first ~400KB; the
            # weight stream dispatches on Sync, anchors+biases on Scalar
            # (the two HWDGE engines dispatch in parallel; each dispatch
            # instruction costs ~0.7us serially on its engine). ----
            def slab_split(pool, t, cols, name, eng, halves=False):
                # whole or half-slab DMAs: keep per-partition lines >= 8KB
                # (fine-grained splits collapse DMA throughput)
                s = pool.tile([128, 8, cols], BF16, name=name)
                if halves:
                    eng.dma_start(out=s[:, 0:4, :], in_=t.ap()[:, 0:4, :])
                    eng.dma_start(out=s[:, 4:8, :], in_=t.ap()[:, 4:8, :])
                else:
                    eng.dma_start(out=s, in_=t.ap())
                return s

            def bias_in(t, name):  # host pre-arranged [128, 8]
                s = cpool.tile([128, 8], F32, name=name)
                nc.scalar.dma_start(out=s, in_=t.ap())
                return s

            def bias_bc(t, name):  # host pre-broadcast [128, D]
                s = cpool.tile([128, D], F32, name=name)
                nc.scalar.dma_start(out=s, in_=t.ap())
                return s

            # K-proj feed first on both queues; then the Q-proj inputs
            # interleaved per feature-chunk (qproj(ct) consumes wlo AND whi
            # from group 0 on) so group 0's scores are never starved.
            wk_sb = slab_split(wearly, wk, D, "wk_sb", nc.sync)
            at_sb = wearly.tile([128, 8, A], BF16, name="at_sb")
            nc.scalar.dma_start(out=at_sb, in_=at.ap())
            wv_sb = slab_split(wearly, wv, D, "wv_sb", nc.sync)
            blo_sb = bias_in(blo, "blo_sb")
            bhi_sb = bias_in(bhi, "bhi_sb")
            bk_sb = bias_in(bk, "bk_sb")
            bv_bc = bias_bc(bv, "bv_bc")
            bo_bc = bias_bc(bo, "bo_bc")
            whi_sb = slab_split(wpool, whi, D, "whi_sb", nc.scalar)
            wlo_sb = slab_split(wpool, wlo, D, "wlo_sb", nc.sync)
            xt_sb = slab_split(wpool, xt, RPC, "xt_sb", nc.sync)
            wo_sb = slab_split(wpool, wo, D, "wo_sb", nc.sync)


            # ---- constants + zero-inits, all on the idle GPSIMD engine ----
            # 0/1 selector for the 1/sums partition broadcast: out rows 0-63
            # take moving row 0, rows 64-127 take moving row 64.
            sel_sb = cpool.tile([65, 128], BF16, name="sel_sb")
            nc.vector.memset(sel_sb, 0.0)
            nc.vector.memset(sel_sb[0:1, 0:64], 1.0)
            nc.vector.memset(sel_sb[64:65, 64:128], 1.0)

            # V slab: [128(a%128), ach, head, 65]; cols 0-63 = V head slice,
            # col 64 = ones (supplies softmax row-sums during AV).
            vaug = kvpool.tile([128, 4, H, DH + 1], BF16, name="vaug")
            nc.vector.memset(vaug, 1.0)

            # fixed parity tiles for the 1/sums chain; rows other than 0/64
            # hold 1.0 forever so the reciprocal/cast stay finite
            sums4s, rcp4s, rcpbfs = [], [], []
            for eo in range(2):
                s4 = cpool.tile([65, 2, 512], F32, name=f"sums4_{eo}")
                nc.vector.memset(s4, 1.0)
                sums4s.append(s4)
                r4 = cpool.tile([65, 2, 512], F32, name=f"rcp4_{eo}")
                rcp4s.append(r4)
                rb = cpool.tile([65, 2, 512], BF16, name=f"rcpbf_{eo}")
                rcpbfs.append(rb)

            qtz = []
            for rc in range(2):
                qt_z0 = qtpool.tile([128, 8, 512], BF16, tag=f"qt0_{rc}",
                                    name=f"qt_z0_{rc}", bufs=1)
                qt_z1 = qtpool.tile([128, 8, 512], BF16, tag=f"qt1_{rc}",
                                    name=f"qt_z1_{rc}", bufs=1)
                nc.vector.memset(qt_z0[64:128, :, :], 0.0)
                nc.vector.memset(qt_z1[0:64, :, :], 0.0)
                qtz.append((qt_z0, qt_z1))

            kt_sb = kvpool.tile([128, 8, A], BF16, name="kt_sb")

            # ---- K^T projection: kt[c, a] = (anchors @ Wk)^T ----
            for ct in range(8):
                pk = psum.tile([128, A], F32, tag="work", name="pk")
                for dt in range(8):
                    nc.tensor.matmul(
                        pk, wk_sb[:, dt, ct * 128:(ct + 1) * 128],
                        at_sb[:, dt, :], start=(dt == 0), stop=(dt == 7))
                nc.vector.tensor_scalar_add(
                    kt_sb[:, ct, :], pk, bk_sb[:, ct:ct + 1])

            # ---- Q^T projection for one head-pair group, written into two
            # zero-padded slabs (z0: odd-head partitions zeroed, z1: even)
            # so score matmuls contract over the full 128 partitions. The pq
            # PSUM tile shares the score tiles' "s" tag/rotation. ----
            def qproj(ct):
                for rc in range(2):
                    wsel = wlo_sb if rc == 0 else whi_sb
                    bsel = blo_sb if rc == 0 else bhi_sb
                    qt_z0, qt_z1 = qtz[rc]
                    pq = psum.tile([128, 512], F32, tag="s", name="pq",
                                   bufs=2)
                    for dt in range(8):
                        nc.tensor.matmul(
                            pq, wsel[:, dt, ct * 128:(ct + 1) * 128],
                            xt_sb[:, dt, rc * 512:(rc + 1) * 512],
                            start=(dt == 0), stop=(dt == 7))
                    nc.vector.tensor_scalar_add(
                        qt_z0[0:64, ct, :], pq[0:64, :], bsel[0:64, ct:ct + 1])
                    nc.vector.tensor_scalar_add(
                        qt_z1[64:128, ct, :], pq[64:128, :],
                        bsel[64:128, ct:ct + 1])

            qproj(0)
            qproj(1)

            # ---- attention, software-pipelined over the 8 head-pair
            # groups (ct): scores+exp run one group ahead of AV, two ahead
            # of the normalization; group ct+1's Q projection is emitted at
            # the tail of stage ct so the PE never crosses a phase barrier.
            # ---- V projection (un-transposed): v[a, c] = anchors @ Wv ----
            for ach in range(4):
                for ch in range(2):
                    pv = psum.tile([128, 512], F32, tag="work", name="pv")
                    for dt in range(8):
                        nc.tensor.matmul(
                            pv, at_sb[:, dt, ach * 128:(ach + 1) * 128],
                            wv_sb[:, dt, ch * 512:(ch + 1) * 512],
                            start=(dt == 0), stop=(dt == 7))
                    pv_v = pv.rearrange("p (hd d) -> p hd d", d=DH)
                    bv_v = bv_bc.rearrange(
                        "p (chd hd d) -> p chd hd d", chd=2, d=DH)[:, ch]
                    nc.vector.tensor_add(
                        vaug[:, ach, ch * 8:(ch + 1) * 8, 0:DH], pv_v, bv_v)

            projstack.close()
            attnstack = ExitStack()
            attnpool = attnstack.enter_context(
                tc.tile_pool(name="attnpool", bufs=1))
            ptpool = attnstack.enter_context(
                tc.tile_pool(name="ptpool", bufs=8))
            tmppool = attnstack.enter_context(
                tc.tile_pool(name="tmppool", bufs=4))
            outpool = attnstack.enter_context(
                tc.tile_pool(name="outpool", bufs=3))
            attnT = attnpool.tile([128, 8, RPC], BF16, name="attnT")

            def vchunk(ach, ch):
                pv = psum.tile([128, 512], F32, tag="work", name="pv")
                for dt in range(8):
                    nc.tensor.matmul(
                        pv, at_sb[:, dt, ach * 128:(ach + 1) * 128],
                        wv_sb[:, dt, ch * 512:(ch + 1) * 512],
                        start=(dt == 0), stop=(dt == 7))
                pv_v = pv.rearrange("p (hd d) -> p hd d", d=DH)
                bv_v = bv_bc.rearrange(
                    "p (chd hd d) -> p chd hd d", chd=2, d=DH)[:, ch]
                nc.vector.tensor_add(
                    vaug[:, ach, ch * 8:(ch + 1) * 8, 0:DH], pv_v, bv_v)

            def score_pair(ct, st, par, rc, half):
                # two score matmuls + one exp for heads {2ct, 2ct+1}
                if half == 0:
                    st["pts"][(par, rc)] = ptpool.tile(
                        [128, 4, 512], BF16, tag="pt", name="pt")
                pt = st["pts"][(par, rc)]
                qt_sb = qtz[rc][par]
                s2 = psum.tile([128, 2, 512], F32, tag="s", name="s2", bufs=2)
                for k in range(2):
                    ach = 2 * half + k
                    nc.tensor.matmul(
                        s2[:, k, :], kt_sb[:, ct, ach * 128:(ach + 1) * 128],
                        qt_sb[:, ct, :], start=True, stop=True)
                nc.scalar.activation(
                    out=pt[:, 2 * half:2 * half + 2, :], in_=s2,
                    func=Exp, scale=SCALE)

            def av_mm(ct, par, st, rc):
                # AV accumulation chain for one row-chunk
                h = 2 * ct + par
                if rc == 0:
                    st[f"pav{par}"] = psum.tile([128, 2, 512], F32,
                                                tag="work", name="pav",
                                                bufs=2)
                pav = st[f"pav{par}"]
                pt = st["pts"][(par, rc)]
                for ach in range(4):
                    nc.tensor.matmul(
                        pav[0:DH + 1, rc, :], vaug[:, ach, h, :],
                        pt[:, ach, :], start=(ach == 0), stop=(ach == 3))
                if rc == 1:
                    row = par * 64
                    if par == 0:
                        st["praw2"] = tmppool.tile([128, 2, 512], BF16,
                                                   tag="praw", name="praw2")
                        nc.scalar.copy(st["praw2"][row:row + DH, :, :],
                                       pav[0:DH, :, :])
                    else:
                        nc.vector.tensor_copy(st["praw2"][row:row + DH, :, :],
                                              pav[0:DH, :, :])
                    nc.vector.tensor_copy(sums4s[ct % 2][row:row + 1, :, :],
                                          pav[DH:DH + 1, :, :])

            def stage_recip(ct, st):
                nc.vector.reciprocal_approx_fast(rcp4s[ct % 2],
                                                 sums4s[ct % 2])
                nc.vector.tensor_copy(rcpbfs[ct % 2], rcp4s[ct % 2])

            def stage_norm(ct, st):
                pav1 = st["pav1"]
                for rcn in range(2):
                    nc.tensor.matmul(
                        pav1[:, rcn, :], sel_sb, rcpbfs[ct % 2][:, rcn, :],
                        start=True, stop=True)
                dst = attnT[:, ct, :].rearrange("p (b r) -> p b r", b=2)
                nc.vector.tensor_mul(dst, st["praw2"], pav1)

            # O-proj partials for the first tiles are emitted inside the
            # pipeline drain so the PE has work while the last groups'
            # normalization chains run; the ones at i==8 borrow the then-idle
            # score-tag PSUM slots.
            pouts_head = {}

            def oproj_partial(rti, nh, tag, upto):
                pout = psum.tile([128, 512], F32, tag=tag, name="pout")
                for ct2 in range(upto):
                    nc.tensor.matmul(
                        pout, attnT[:, ct2, rti * 128:(rti + 1) * 128],
                        wo_sb[:, ct2, nh * 512:(nh + 1) * 512],
                        start=(ct2 == 0), stop=False)
                pouts_head[(rti, nh)] = (pout, upto)

            # Main pipeline: per stage i, the 8 exp-gated score pairs of
            # group i are interleaved with independent PE fill (AV of group
            # i-1, Q proj of group i+2, V projection at stage 0) so the PE
            # never waits on the scalar engine's exp stream.
            sts = {}
            for i in range(10):
                fills = []
                if 1 <= i <= 8:
                    st_p = sts[i - 1]
                    fills.append(lambda st_p=st_p, i=i: av_mm(i - 2 + 1, 0, st_p, 0))
                    fills.append(lambda st_p=st_p, i=i: av_mm(i - 2 + 1, 0, st_p, 1))
                if i <= 5:
                    fills.append(lambda i=i: qproj(i + 2))
                if i < 8:
                    st = sts[i] = {"pts": {}}
                    pairs = [(par, rc, half) for par in range(2)
                             for rc in range(2) for half in range(2)]
                    fi = 0
                    for pi, (par, rc, half) in enumerate(pairs):
                        score_pair(i, st, par, rc, half)
                        if pi >= 1 and fi < len(fills) and pi < 6:
                            fills[fi](); fi += 1
                    while fi < len(fills):
                        fills[fi](); fi += 1
                else:
                    fi = 0
                    while fi < len(fills):
                        fills[fi](); fi += 1
                if i == 9:
                    oproj_partial(1, 0, "work", 7)
                if 2 <= i <= 9:
                    stage_recip(i - 2, sts[i - 2])
                    stage_norm(i - 2, sts[i - 2])
                if 1 <= i <= 8:
                    st_p = sts[i - 1]
                    av_mm(i - 1, 1, st_p, 0)
                    av_mm(i - 1, 1, st_p, 1)
                if i == 8:
                    oproj_partial(0, 0, "s", 7)
                    oproj_partial(0, 1, "s", 7)

            # ---- output projection ----
            oproj_partial(1, 1, "work", 0)
            for rti in range(8):
                for nh in range(2):
                    if (rti, nh) in pouts_head:
                        pout, upto = pouts_head[(rti, nh)]
                        for ct2 in range(upto, 8):
                            nc.tensor.matmul(
                                pout, attnT[:, ct2, rti * 128:(rti + 1) * 128],
                                wo_sb[:, ct2, nh * 512:(nh + 1) * 512],
                                start=(ct2 == 0), stop=(ct2 == 7))
                    else:
                        pout = psum.tile([128, 512], F32, tag="work",
                                         name="pout")
                        for ct2 in range(8):
                            nc.tensor.matmul(
                                pout, attnT[:, ct2, rti * 128:(rti + 1) * 128],
                                wo_sb[:, ct2, nh * 512:(nh + 1) * 512],
                                start=(ct2 == 0), stop=(ct2 == 7))
                    out_t = outpool.tile([128, 512], F32, tag="out",
                                         name="out_t")
                    nc.vector.tensor_add(out_t, pout,
                                         bo_bc[:, nh * 512:(nh + 1) * 512])
                    nc.sync.dma_start(
                        out=out.ap()[rti * 128:(rti + 1) * 128,
                                     nh * 512:(nh + 1) * 512],
                        in_=out_t)
            attnstack.close()

    nc.compile()
    return nc


def _swz(a):
    """[1024, cols] -> [128, 8, cols] with row r -> (r % 128, r // 128)."""
    return np.ascontiguousarray(
        a.reshape(8, 128, -1).transpose(1, 0, 2))


def _make_in_maps(x, Wq, bq, Wk, bk, Wv, bv, Wqt, bqt, Wo, bo):
    x = np.asarray(x, dtype=np.float32)
    bf = ml_dtypes.bfloat16

    wq_b = np.ascontiguousarray(np.asarray(Wq, np.float32).astype(bf))
    wqt_b = np.ascontiguousarray(np.asarray(Wqt, np.float32).astype(bf))
    wk_b = np.ascontiguousarray(np.asarray(Wk, np.float32).astype(bf))
    wv_b = np.ascontiguousarray(np.asarray(Wv, np.float32).astype(bf))
    wo_b = np.ascontiguousarray(np.asarray(Wo, np.float32).astype(bf))
    colmaj = lambda v: np.ascontiguousarray(
        np.asarray(v, np.float32).reshape(8, 128).T)
    bq, bqt, bk = map(colmaj, (bq, bqt, bk))
    bv = np.ascontiguousarray(
        np.broadcast_to(np.asarray(bv, np.float32), (128, D)))
    bo = np.ascontiguousarray(
        np.broadcast_to(np.asarray(bo, np.float32), (128, D)))

    wq_sw, wqt_sw = _swz(wq_b), _swz(wqt_b)
    wk_sw, wv_sw, wo_sw = _swz(wk_b), _swz(wv_b), _swz(wo_b)
    at_sw = [_swz(x[b, :A, :].T.astype(bf)) for b in range(B)]
    in_maps = []
    for c in range(NCORES):
        b, q = divmod(c, 4)
        rows = x[b, q * RPC:(q + 1) * RPC, :]
        in_maps.append({
            "xt": _swz(rows.T.astype(bf)),
            "at": at_sw[b],
            "wlo": wq_sw if q == 0 else wqt_sw,
            "whi": wqt_sw,
            "wk": wk_sw, "wv": wv_sw, "wo": wo_sw,
            "blo": bq if q == 0 else bqt, "bhi": bqt,
            "bk": bk, "bv": bv, "bo": bo,
        })
    return in_maps


def kernel(x, Wq, bq, Wk, bk, Wv, bv, Wqt, bqt, Wo, bo, num_anchor_tokens):
    assert int(num_anchor_tokens) == A
    if "nc" not in _CACHE:
        _CACHE["nc"] = _build()
    nc = _CACHE["nc"]

    in_maps = _make_in_maps(x, Wq, bq, Wk, bk, Wv, bv, Wqt, bqt, Wo, bo)
    res = bass_utils.run_bass_kernel_spmd(
        nc, in_maps, core_ids=list(range(NCORES)))
    out = np.empty((B, S, D), np.float32)
    for c in range(NCORES):
        b, q = divmod(c, 4)
        out[b, q * RPC:(q + 1) * RPC, :] = res.results[c]["out"]
    return out


# revision 31
# speedup vs baseline: 1.0495x; 1.0019x over previous
"""AnchorAttention distributed Bass kernel for 8 TRN2 NeuronCores.

Reference computation (B=2, S=4096, D=1024, H=16, Dh=64, A=512):
  anchors = x[:, :A];  queries = x[:, A:]
  anchor_q/k/v = split_heads(anchors @ Wq/Wk/Wv + b)
  query_q      = split_heads(queries @ Wqt + bqt)
  combined_q   = concat([anchor_q, query_q], axis=2)       # [B,H,S,Dh]
  out  = softmax(combined_q @ anchor_k^T / sqrt(Dh)) @ anchor_v
  out  = merge_heads(out) @ Wo + bo

Sharding: the B*S = 8192 token rows are split into 8 chunks of 1024 rows
(core c -> batch c//4, rows (c%4)*1024 ...). Each core duplicates its
batch's anchor K/V projections, computes Q for its own rows (Wq for the
anchor-region rows, Wqt for query rows), attention over the 512 anchors
for all 16 heads, and the output projection for its rows. The output is a
pure concatenation: no collectives.

Layout: everything is kept transposed ([feature, row]) so each matmul
contracts over the partition dim with zero on-chip transposes; the final
output projection naturally lands un-transposed [row, feature] for DMA
out. Host pre-transposes/pre-casts inputs to bf16 (f32 accumulation in
PSUM). Softmax row-sums come free via an extra all-ones column appended
to V; no max-subtraction is needed (softmax is shift-invariant and the
scores are ~N(0,1)).

Schedule: the PE instruction stream is issue-bound (~0.26us per 512-col
matmul), so the kernel is organized to keep it saturated end to end.
Input DMAs are split per feature-chunk across the two HWDGE dispatch
engines (weights on Sync, anchors/biases on Scalar) so the first K-proj
matmul starts ~2.5us in instead of waiting for whole slabs behind a
serial dispatch queue. Q projection is streamed per head-pair INSIDE the
attention pipeline (group ct+1's tiles are emitted at the tail of stage
ct) so no phase barrier exists anywhere; O-proj partial accumulations
fill the pipeline drain. 1/sums are computed by the DVE fast reciprocal
directly from the PSUM sums rows into fixed parity tiles, cast on the
GPSIMD engine, and partition-broadcast by a single [65,128] 0/1-selector
matmul per row-chunk written into the already-evacuated rows of the
pair's second AV PSUM tile; one mixed-partition-base DVE multiply then
writes the normalized attn^T slab. PSUM evacuations split across the
Scalar/Vector engines; all memsets run on the idle GPSIMD engine.
"""

import numpy as np
import ml_dtypes

import concourse.bass as bass
import concourse.tile as tile
from concourse import bacc, mybir
from concourse import bass_utils

BF16 = mybir.dt.bfloat16
F32 = mybir.dt.float32
B, S, D = 2, 4096, 1024
H, DH = 16, 64
A = 512                  # num_anchor_tokens (asserted at runtime)
RPC = 1024               # rows per core
NCORES = 8
SCALE = 1.0 / np.sqrt(float(DH))

_CACHE = {}


def _build():
    """Build + compile the per-core Bass graph (identical on all cores)."""
    nc = bacc.Bacc("TRN2", target_bir_lowering=False, debug=False)

    xt = nc.dram_tensor("xt", [128, 8, RPC], BF16, kind="ExternalInput")   # rows^T swizzled
    at = nc.dram_tensor("at", [128, 8, A], BF16, kind="ExternalInput")     # anchors^T swizzled
    wlo = nc.dram_tensor("wlo", [128, 8, D], BF16, kind="ExternalInput")   # Q weight rows 0-511
    whi = nc.dram_tensor("whi", [128, 8, D], BF16, kind="ExternalInput")   # Q weight rows 512-1023
    wk = nc.dram_tensor("wk", [128, 8, D], BF16, kind="ExternalInput")
    wv = nc.dram_tensor("wv", [128, 8, D], BF16, kind="ExternalInput")
    wo = nc.dram_tensor("wo", [128, 8, D], BF16, kind="ExternalInput")
    blo = nc.dram_tensor("blo", [128, 8], F32, kind="ExternalInput")
    bhi = nc.dram_tensor("bhi", [128, 8], F32, kind="ExternalInput")
    bk = nc.dram_tensor("bk", [128, 8], F32, kind="ExternalInput")
    bv = nc.dram_tensor("bv", [128, D], BF16, kind="ExternalInput")  # pre-broadcast
    bo = nc.dram_tensor("bo", [128, D], BF16, kind="ExternalInput")  # pre-broadcast
    out = nc.dram_tensor("out", [RPC, D], F32, kind="ExternalOutput")

    Exp = mybir.ActivationFunctionType.Exp

    from contextlib import ExitStack

    with tile.TileContext(nc) as tc:
        with tc.tile_pool(name="wpool", bufs=1) as wpool, \
             tc.tile_pool(name="cpool", bufs=1) as cpool, \
             tc.tile_pool(name="kvpool", bufs=1) as kvpool, \
             tc.tile_pool(name="qtpool", bufs=2) as qtpool, \
             tc.tile_pool(name="psum", bufs=2, space="PSUM") as psum:
            # wk/wv/at live only through the K/V projections; their pool
            # closes before the attention pools open so the attention
            # working set reuses their SBUF space.
            attnstack = ExitStack()
            attnpool = attnstack.enter_context(
                tc.tile_pool(name="attnpool", bufs=1))
            ptpool = attnstack.enter_context(
                tc.tile_pool(name="ptpool", bufs=6))
            tmppool = attnstack.enter_context(
                tc.tile_pool(name="tmppool", bufs=3))
            outpool = attnstack.enter_context(
                tc.tile_pool(name="outpool", bufs=2))
            projstack = ExitStack()
            wearly = projstack.enter_context(
                tc.tile_pool(name="wearly", bufs=1))
            wkstack = ExitStack()
            wkpool = wkstack.enter_context(
                tc.tile_pool(name="wkpool", bufs=1))

            # ---- input DMAs. Big slabs are split per feature-chunk (dt) so
            # subtile deps let consumers start after the first ~400KB; the
            # weight stream dispatches on Sync, anchors+biases on Scalar
            # (the two HWDGE engines dispatch in parallel; each dispatch
            # instruction costs ~0.7us serially on its engine). ----
            def slab_split(pool, t, cols, name, eng, halves=False):
                # whole or half-slab DMAs: keep per-partition lines >= 8KB
                # (fine-grained splits collapse DMA throughput)
                s = pool.tile([128, 8, cols], BF16, name=name)
                if halves:
                    eng.dma_start(out=s[:, 0:4, :], in_=t.ap()[:, 0:4, :])
                    eng.dma_start(out=s[:, 4:8, :], in_=t.ap()[:, 4:8, :])
                else:
                    eng.dma_start(out=s, in_=t.ap())
                return s

            def bias_in(t, name):  # host pre-arranged [128, 8]
                s = cpool.tile([128, 8], F32, name=name)
                nc.scalar.dma_start(out=s, in_=t.ap())
                return s

            def bias_bc(t, name):  # host pre-broadcast [128, D] bf16
                s = cpool.tile([128, D], BF16, name=name)
                nc.scalar.dma_start(out=s, in_=t.ap())
                return s

            # K-proj feed first on both queues; then the Q-proj inputs
            # interleaved per feature-chunk (qproj(ct) consumes wlo AND whi
            # from group 0 on) so group 0's scores are never starved.
            wk_sb = slab_split(wkpool, wk, D, "wk_sb", nc.sync)
            at_sb = wearly.tile([128, 8, A], BF16, name="at_sb")
            nc.scalar.dma_start(out=at_sb, in_=at.ap())
            wv_sb = slab_split(wearly, wv, D, "wv_sb", nc.sync)
            blo_sb = bias_in(blo, "blo_sb")
            bhi_sb = bias_in(bhi, "bhi_sb")
            bk_sb = bias_in(bk, "bk_sb")
            bv_bc = bias_bc(bv, "bv_bc")
            bo_bc = bias_bc(bo, "bo_bc")
            whi_sb = slab_split(wpool, whi, D, "whi_sb", nc.scalar)
            wlo_sb = slab_split(wpool, wlo, D, "wlo_sb", nc.sync)
            xt_sb = slab_split(wpool, xt, RPC, "xt_sb", nc.sync)
            hold = {}


            # ---- constants + zero-inits, all on the idle GPSIMD engine ----
            # 0/1 selector for the 1/sums partition broadcast: out rows 0-63
            # take moving row 0, rows 64-127 take moving row 64.
            sel_sb = cpool.tile([65, 128], BF16, name="sel_sb")
            nc.vector.memset(sel_sb, 0.0)
            nc.vector.memset(sel_sb[0:1, 0:64], 1.0)
            nc.vector.memset(sel_sb[64:65, 64:128], 1.0)

            # V slab: [128(a%128), ach, head, 65]; cols 0-63 = V head slice,
            # col 64 = ones (supplies softmax row-sums during AV).
            vaug = kvpool.tile([128, 4, H, DH + 1], BF16, name="vaug")
            nc.vector.memset(vaug, 1.0)

            # fixed parity tiles for the 1/sums chain; rows other than 0/64
            # hold 1.0 forever so the reciprocal/cast stay finite
            sums4s, rcpbfs = [], []
            rcp4 = cpool.tile([65, 2, 512], F32, name="rcp4")
            for eo in range(2):
                s4 = cpool.tile([65, 2, 512], F32, name=f"sums4_{eo}")
                nc.vector.memset(s4, 1.0)
                sums4s.append(s4)
                rb = cpool.tile([65, 2, 512], BF16, name=f"rcpbf_{eo}")
                rcpbfs.append(rb)

            qtz = []
            for rc in range(2):
                qt_z0 = qtpool.tile([128, 8, 512], BF16, tag=f"qt0_{rc}",
                                    name=f"qt_z0_{rc}", bufs=1)
                qt_z1 = qtpool.tile([128, 8, 512], BF16, tag=f"qt1_{rc}",
                                    name=f"qt_z1_{rc}", bufs=1)
                nc.vector.memset(qt_z0[64:128, :, :], 0.0)
                nc.vector.memset(qt_z1[0:64, :, :], 0.0)
                qtz.append((qt_z0, qt_z1))

            kt_sb = kvpool.tile([128, 8, A], BF16, name="kt_sb")

            # ---- K^T projection: kt[c, a] = (anchors @ Wk)^T ----
            for ct in range(8):
                pk = psum.tile([128, A], F32, tag="work", name="pk")
                for dt in range(8):
                    nc.tensor.matmul(
                        pk, wk_sb[:, dt, ct * 128:(ct + 1) * 128],
                        at_sb[:, dt, :], start=(dt == 0), stop=(dt == 7))
                nc.vector.tensor_scalar_add(
                    kt_sb[:, ct, :], pk, bk_sb[:, ct:ct + 1])

            # ---- Q^T projection for one head-pair group, written into two
            # zero-padded slabs (z0: odd-head partitions zeroed, z1: even)
            # so score matmuls contract over the full 128 partitions. The pq
            # PSUM tile shares the score tiles' "s" tag/rotation. ----
            def qproj(ct):
                for rc in range(2):
                    wsel = wlo_sb if rc == 0 else whi_sb
                    bsel = blo_sb if rc == 0 else bhi_sb
                    qt_z0, qt_z1 = qtz[rc]
                    pq = psum.tile([128, 512], F32, tag="s", name="pq",
                                   bufs=2)
                    for dt in range(8):
                        nc.tensor.matmul(
                            pq, wsel[:, dt, ct * 128:(ct + 1) * 128],
                            xt_sb[:, dt, rc * 512:(rc + 1) * 512],
                            start=(dt == 0), stop=(dt == 7))
                    nc.vector.tensor_scalar_add(
                        qt_z0[0:64, ct, :], pq[0:64, :], bsel[0:64, ct:ct + 1])
                    nc.vector.tensor_scalar_add(
                        qt_z1[64:128, ct, :], pq[64:128, :],
                        bsel[64:128, ct:ct + 1])

            qproj(0)
            qproj(1)

            # ---- attention, software-pipelined over the 8 head-pair
            # groups (ct): scores+exp run one group ahead of AV, two ahead
            # of the normalization; group ct+1's Q projection is emitted at
            # the tail of stage ct so the PE never crosses a phase barrier.
            wkstack.close()
            attnT = attnpool.tile([128, 8, RPC], BF16, name="attnT")

            def vchunk(ach, ch):
                pv = psum.tile([128, 512], F32, tag="work", name="pv")
                for dt in range(8):
                    nc.tensor.matmul(
                        pv, at_sb[:, dt, ach * 128:(ach + 1) * 128],
                        wv_sb[:, dt, ch * 512:(ch + 1) * 512],
                        start=(dt == 0), stop=(dt == 7))
                pv_v = pv.rearrange("p (hd d) -> p hd d", d=DH)
                bv_v = bv_bc.rearrange(
                    "p (chd hd d) -> p chd hd d", chd=2, d=DH)[:, ch]
                nc.vector.tensor_add(
                    vaug[:, ach, ch * 8:(ch + 1) * 8, 0:DH], pv_v, bv_v)

            def score_pair(ct, st, par, rc, half):
                # two score matmuls + one exp for heads {2ct, 2ct+1}
                if half == 0:
                    st["pts"][(par, rc)] = ptpool.tile(
                        [128, 4, 512], BF16, tag="pt", name="pt")
                pt = st["pts"][(par, rc)]
                qt_sb = qtz[rc][par]
                s2 = psum.tile([128, 2, 512], F32, tag="s", name="s2", bufs=2)
                for k in range(2):
                    ach = 2 * half + k
                    nc.tensor.matmul(
                        s2[:, k, :], kt_sb[:, ct, ach * 128:(ach + 1) * 128],
                        qt_sb[:, ct, :], start=True, stop=True)
                nc.scalar.activation(
                    out=pt[:, 2 * half:2 * half + 2, :], in_=s2,
                    func=Exp, scale=SCALE)

            def av_mm(ct, par, st, rc):
                # AV accumulation chain for one row-chunk
                h = 2 * ct + par
                if rc == 0:
                    st[f"pav{par}"] = psum.tile([128, 2, 512], F32,
                                                tag="work", name="pav",
                                                bufs=2)
                pav = st[f"pav{par}"]
                pt = st["pts"][(par, rc)]
                for ach in range(4):
                    nc.tensor.matmul(
                        pav[0:DH + 1, rc, :], vaug[:, ach, h, :],
                        pt[:, ach, :], start=(ach == 0), stop=(ach == 3))
                if rc == 1:
                    row = par * 64
                    if par == 0:
                        st["praw2"] = tmppool.tile([128, 2, 512], BF16,
                                                   tag="praw", name="praw2")
                        nc.scalar.copy(st["praw2"][row:row + DH, :, :],
                                       pav[0:DH, :, :])
                    else:
                        nc.vector.tensor_copy(st["praw2"][row:row + DH, :, :],
                                              pav[0:DH, :, :])
                    nc.vector.tensor_copy(sums4s[ct % 2][row:row + 1, :, :],
                                          pav[DH:DH + 1, :, :])

            def stage_recip(ct, st):
                nc.vector.reciprocal_approx_fast(rcp4, sums4s[ct % 2])
                nc.vector.tensor_copy(rcpbfs[ct % 2], rcp4)

            def stage_norm(ct, st):
                pav1 = st["pav1"]
                for rcn in range(2):
                    nc.tensor.matmul(
                        pav1[:, rcn, :], sel_sb, rcpbfs[ct % 2][:, rcn, :],
                        start=True, stop=True)
                dst = attnT[:, ct, :].rearrange("p (b r) -> p b r", b=2)
                nc.vector.tensor_mul(dst, st["praw2"], pav1)

            # O-proj partials for the first tiles are emitted inside the
            # pipeline drain so the PE has work while the last groups'
            # normalization chains run; the ones at i==8 borrow the then-idle
            # score-tag PSUM slots.
            pouts_head = {}

            def oproj_partial(rti, nh, tag, upto):
                pout = psum.tile([128, 512], F32, tag=tag, name="pout")
                for ct2 in range(upto):
                    nc.tensor.matmul(
                        pout, attnT[:, ct2, rti * 128:(rti + 1) * 128],
                        hold["wo_sb"][:, ct2, nh * 512:(nh + 1) * 512],
                        start=(ct2 == 0), stop=False)
                pouts_head[(rti, nh)] = (pout, upto)

            # Main pipeline: per stage i, the 8 exp-gated score pairs of
            # group i are interleaved with independent PE fill (AV of group
            # i-1, Q proj of group i+2, V projection at stage 0) so the PE
            # never waits on the scalar engine's exp stream.
            sts = {}
            for i in range(10):
                if i == 1:
                    projstack.close()
                    wopool = attnstack.enter_context(
                        tc.tile_pool(name="wopool", bufs=1))
                    hold["wo_sb"] = slab_split(wopool, wo, D, "wo_sb", nc.sync)
                fills = []
                if i == 0:
                    for ach in range(4):
                        for ch in range(2):
                            fills.append(lambda ach=ach, ch=ch: vchunk(ach, ch))
                if 1 <= i <= 8:
                    st_p = sts[i - 1]
                    fills.append(lambda st_p=st_p, i=i: av_mm(i - 2 + 1, 0, st_p, 0))
                    fills.append(lambda st_p=st_p, i=i: av_mm(i - 2 + 1, 0, st_p, 1))
                if i <= 5:
                    fills.append(lambda i=i: qproj(i + 2))
                if i < 8:
                    st = sts[i] = {"pts": {}}
                    pairs = [(par, rc, half) for par in range(2)
                             for rc in range(2) for half in range(2)]
                    fi = 0
                    for pi, (par, rc, half) in enumerate(pairs):
                        score_pair(i, st, par, rc, half)
                        if pi >= 1 and fi < len(fills) and pi < 6:
                            fills[fi](); fi += 1
                    while fi < len(fills):
                        fills[fi](); fi += 1
                else:
                    fi = 0
                    while fi < len(fills):
                        fills[fi](); fi += 1
                if i == 9:
                    oproj_partial(1, 0, "work", 7)
                if 2 <= i <= 9:
                    stage_recip(i - 2, sts[i - 2])
                    stage_norm(i - 2, sts[i - 2])
                if 1 <= i <= 8:
                    st_p = sts[i - 1]
                    av_mm(i - 1, 1, st_p, 0)
                    av_mm(i - 1, 1, st_p, 1)
                if i == 8:
                    oproj_partial(0, 0, "s", 7)
                    oproj_partial(0, 1, "s", 7)

            # ---- output projection ----
            oproj_partial(1, 1, "work", 0)
            for rti in range(8):
                for nh in range(2):
                    if (rti, nh) in pouts_head:
                        pout, upto = pouts_head[(rti, nh)]
                        for ct2 in range(upto, 8):
                            nc.tensor.matmul(
                                pout, attnT[:, ct2, rti * 128:(rti + 1) * 128],
                                hold["wo_sb"][:, ct2, nh * 512:(nh + 1) * 512],
                                start=(ct2 == 0), stop=(ct2 == 7))
                    else:
                        pout = psum.tile([128, 512], F32, tag="work",
                                         name="pout")
                        for ct2 in range(8):
                            nc.tensor.matmul(
                                pout, attnT[:, ct2, rti * 128:(rti + 1) * 128],
                                hold["wo_sb"][:, ct2, nh * 512:(nh + 1) * 512],
                                start=(ct2 == 0), stop=(ct2 == 7))
                    out_t = outpool.tile([128, 512], F32, tag="out",
                                         name="out_t")
                    nc.vector.tensor_add(out_t, pout,
                                         bo_bc[:, nh * 512:(nh + 1) * 512])
                    nc.sync.dma_start(
                        out=out.ap()[rti * 128:(rti + 1) * 128,
                                     nh * 512:(nh + 1) * 512],
                        in_=out_t)
            attnstack.close()

    nc.compile()
    return nc


def _swz(a):
    """[1024, cols] -> [128, 8, cols] with row r -> (r % 128, r // 128)."""
    return np.ascontiguousarray(
        a.reshape(8, 128, -1).transpose(1, 0, 2))


def _make_in_maps(x, Wq, bq, Wk, bk, Wv, bv, Wqt, bqt, Wo, bo):
    x = np.asarray(x, dtype=np.float32)
    bf = ml_dtypes.bfloat16

    wq_b = np.ascontiguousarray(np.asarray(Wq, np.float32).astype(bf))
    wqt_b = np.ascontiguousarray(np.asarray(Wqt, np.float32).astype(bf))
    wk_b = np.ascontiguousarray(np.asarray(Wk, np.float32).astype(bf))
    wv_b = np.ascontiguousarray(np.asarray(Wv, np.float32).astype(bf))
    wo_b = np.ascontiguousarray(np.asarray(Wo, np.float32).astype(bf))
    colmaj = lambda v: np.ascontiguousarray(
        np.asarray(v, np.float32).reshape(8, 128).T)
    bq, bqt, bk = map(colmaj, (bq, bqt, bk))
    bv = np.ascontiguousarray(
        np.broadcast_to(np.asarray(bv, np.float32).astype(bf), (128, D)))
    bo = np.ascontiguousarray(
        np.broadcast_to(np.asarray(bo, np.float32).astype(bf), (128, D)))

    wq_sw, wqt_sw = _swz(wq_b), _swz(wqt_b)
    wk_sw, wv_sw, wo_sw = _swz(wk_b), _swz(wv_b), _swz(wo_b)
    at_sw = [_swz(x[b, :A, :].T.astype(bf)) for b in range(B)]
    in_maps = []
    for c in range(NCORES):
        b, q = divmod(c, 4)
        rows = x[b, q * RPC:(q + 1) * RPC, :]
        in_maps.append({
            "xt": _swz(rows.T.astype(bf)),
            "at": at_sw[b],
            "wlo": wq_sw if q == 0 else wqt_sw,
            "whi": wqt_sw,
            "wk": wk_sw, "wv": wv_sw, "wo": wo_sw,
            "blo": bq if q == 0 else bqt, "bhi": bqt,
            "bk": bk, "bv": bv, "bo": bo,
        })
    return in_maps


def kernel(x, Wq, bq, Wk, bk, Wv, bv, Wqt, bqt, Wo, bo, num_anchor_tokens):
    assert int(num_anchor_tokens) == A
    if "nc" not in _CACHE:
        _CACHE["nc"] = _build()
    nc = _CACHE["nc"]

    in_maps = _make_in_maps(x, Wq, bq, Wk, bk, Wv, bv, Wqt, bqt, Wo, bo)
    res = bass_utils.run_bass_kernel_spmd(
        nc, in_maps, core_ids=list(range(NCORES)))
    out = np.empty((B, S, D), np.float32)
    for c in range(NCORES):
        b, q = divmod(c, 4)
        out[b, q * RPC:(q + 1) * RPC, :] = res.results[c]["out"]
    return out


# revision 34
# speedup vs baseline: 1.0508x; 1.0012x over previous
"""AnchorAttention distributed Bass kernel for 8 TRN2 NeuronCores.

Reference computation (B=2, S=4096, D=1024, H=16, Dh=64, A=512):
  anchors = x[:, :A];  queries = x[:, A:]
  anchor_q/k/v = split_heads(anchors @ Wq/Wk/Wv + b)
  query_q      = split_heads(queries @ Wqt + bqt)
  combined_q   = concat([anchor_q, query_q], axis=2)       # [B,H,S,Dh]
  out  = softmax(combined_q @ anchor_k^T / sqrt(Dh)) @ anchor_v
  out  = merge_heads(out) @ Wo + bo

Sharding: the B*S = 8192 token rows are split into 8 chunks of 1024 rows
(core c -> batch c//4, rows (c%4)*1024 ...). Each core duplicates its
batch's anchor K/V projections, computes Q for its own rows (Wq for the
anchor-region rows, Wqt for query rows), attention over the 512 anchors
for all 16 heads, and the output projection for its rows. The output is a
pure concatenation: no collectives.

Layout: everything is kept transposed ([feature, row]) so each matmul
contracts over the partition dim with zero on-chip transposes; the final
output projection naturally lands un-transposed [row, feature] for DMA
out. Host pre-transposes/pre-casts inputs to bf16 (f32 accumulation in
PSUM). Softmax row-sums come free via an extra all-ones column appended
to V; no max-subtraction is needed (softmax is shift-invariant and the
scores are ~N(0,1)).

Schedule: the PE instruction stream is issue-bound (~0.26us per 512-col
matmul), so the kernel is organized to keep it saturated end to end.
Input DMAs are split per feature-chunk across the two HWDGE dispatch
engines (weights on Sync, anchors/biases on Scalar) so the first K-proj
matmul starts ~2.5us in instead of waiting for whole slabs behind a
serial dispatch queue. Q projection is streamed per head-pair INSIDE the
attention pipeline (group ct+1's tiles are emitted at the tail of stage
ct) so no phase barrier exists anywhere; O-proj partial accumulations
fill the pipeline drain. 1/sums are computed by the DVE fast reciprocal
directly from the PSUM sums rows into fixed parity tiles, cast on the
GPSIMD engine, and partition-broadcast by a single [65,128] 0/1-selector
matmul per row-chunk written into the already-evacuated rows of the
pair's second AV PSUM tile; one mixed-partition-base DVE multiply then
writes the normalized attn^T slab. PSUM evacuations split across the
Scalar/Vector engines; all memsets run on the idle GPSIMD engine.
"""

import numpy as np
import ml_dtypes

import concourse.bass as bass
import concourse.tile as tile
from concourse import bacc, mybir
from concourse import bass_utils

BF16 = mybir.dt.bfloat16
F32 = mybir.dt.float32
B, S, D = 2, 4096, 1024
H, DH = 16, 64
A = 512                  # num_anchor_tokens (asserted at runtime)
RPC = 1024               # rows per core
NCORES = 8
SCALE = 1.0 / np.sqrt(float(DH))

_CACHE = {}


def _build():
    """Build + compile the per-core Bass graph (identical on all cores)."""
    nc = bacc.Bacc("TRN2", target_bir_lowering=False, debug=False)

    xt = nc.dram_tensor("xt", [128, 8, RPC], BF16, kind="ExternalInput")   # rows^T swizzled
    at = nc.dram_tensor("at", [128, 8, A], BF16, kind="ExternalInput")     # anchors^T swizzled
    wlo = nc.dram_tensor("wlo", [128, 8, D], BF16, kind="ExternalInput")   # Q weight rows 0-511
    whi = nc.dram_tensor("whi", [128, 8, D], BF16, kind="ExternalInput")   # Q weight rows 512-1023
    wk = nc.dram_tensor("wk", [128, 8, D], BF16, kind="ExternalInput")
    wv = nc.dram_tensor("wv", [128, 8, D], BF16, kind="ExternalInput")
    wo = nc.dram_tensor("wo", [128, 8, D], BF16, kind="ExternalInput")
    blo = nc.dram_tensor("blo", [128, 8], F32, kind="ExternalInput")
    bhi = nc.dram_tensor("bhi", [128, 8], F32, kind="ExternalInput")
    bk = nc.dram_tensor("bk", [128, 8], F32, kind="ExternalInput")
    bv = nc.dram_tensor("bv", [128, D], BF16, kind="ExternalInput")  # pre-broadcast
    bo = nc.dram_tensor("bo", [128, D], BF16, kind="ExternalInput")  # pre-broadcast
    out = nc.dram_tensor("out", [RPC, D], F32, kind="ExternalOutput")

    Exp = mybir.ActivationFunctionType.Exp

    from contextlib import ExitStack

    with tile.TileContext(nc) as tc:
        with tc.tile_pool(name="wpool", bufs=1) as wpool, \
             tc.tile_pool(name="cpool", bufs=1) as cpool, \
             tc.tile_pool(name="kvpool", bufs=1) as kvpool, \
             tc.tile_pool(name="qtpool", bufs=2) as qtpool, \
             tc.tile_pool(name="psum", bufs=2, space="PSUM") as psum:
            # wk/wv/at live only through the K/V projections; their pool
            # closes before the attention pools open so the attention
            # working set reuses their SBUF space.
            attnstack = ExitStack()
            attnpool = attnstack.enter_context(
                tc.tile_pool(name="attnpool", bufs=1))
            ptpool = attnstack.enter_context(
                tc.tile_pool(name="ptpool", bufs=6))
            tmppool = attnstack.enter_context(
                tc.tile_pool(name="tmppool", bufs=3))
            outpool = attnstack.enter_context(
                tc.tile_pool(name="outpool", bufs=2))
            projstack = ExitStack()
            wearly = projstack.enter_context(
                tc.tile_pool(name="wearly", bufs=1))
            wkstack = ExitStack()
            wkpool = wkstack.enter_context(
                tc.tile_pool(name="wkpool", bufs=1))

            # ---- input DMAs. Big slabs are split per feature-chunk (dt) so
            # subtile deps let consumers start after the first ~400KB; the
            # weight stream dispatches on Sync, anchors+biases on Scalar
            # (the two HWDGE engines dispatch in parallel; each dispatch
            # instruction costs ~0.7us serially on its engine). ----
            def slab_split(pool, t, cols, name, eng, halves=False):
                # whole or half-slab DMAs: keep per-partition lines >= 8KB
                # (fine-grained splits collapse DMA throughput)
                s = pool.tile([128, 8, cols], BF16, name=name)
                if halves:
                    eng.dma_start(out=s[:, 0:4, :], in_=t.ap()[:, 0:4, :])
                    eng.dma_start(out=s[:, 4:8, :], in_=t.ap()[:, 4:8, :])
                else:
                    eng.dma_start(out=s, in_=t.ap())
                return s

            def bias_in(t, name):  # host pre-arranged [128, 8]
                s = cpool.tile([128, 8], F32, name=name)
                nc.scalar.dma_start(out=s, in_=t.ap())
                return s

            def bias_bc(t, name):  # host pre-broadcast [128, D] bf16
                s = cpool.tile([128, D], BF16, name=name)
                nc.scalar.dma_start(out=s, in_=t.ap())
                return s

            # K-proj feed first on both queues; then the Q-proj inputs
            # interleaved per feature-chunk (qproj(ct) consumes wlo AND whi
            # from group 0 on) so group 0's scores are never starved.
            wk_sb = slab_split(wkpool, wk, D, "wk_sb", nc.sync)
            at_sb = wearly.tile([128, 8, A], BF16, name="at_sb")
            nc.scalar.dma_start(out=at_sb, in_=at.ap())
            wv_sb = slab_split(wearly, wv, D, "wv_sb", nc.sync)
            blo_sb = bias_in(blo, "blo_sb")
            bhi_sb = bias_in(bhi, "bhi_sb")
            bk_sb = bias_in(bk, "bk_sb")
            bv_bc = bias_bc(bv, "bv_bc")
            bo_bc = bias_bc(bo, "bo_bc")
            whi_sb = slab_split(wpool, whi, D, "whi_sb", nc.scalar)
            wlo_sb = slab_split(wpool, wlo, D, "wlo_sb", nc.sync)
            xt_sb = slab_split(wpool, xt, RPC, "xt_sb", nc.sync)
            hold = {}


            # ---- constants + zero-inits, all on the idle GPSIMD engine ----
            # 0/1 selector for the 1/sums partition broadcast: out rows 0-63
            # take moving row 0, rows 64-127 take moving row 64.
            sel_sb = cpool.tile([65, 128], BF16, name="sel_sb")
            nc.vector.memset(sel_sb, 0.0)
            nc.vector.memset(sel_sb[0:1, 0:64], 1.0)
            nc.vector.memset(sel_sb[64:65, 64:128], 1.0)

            # V slab: [128(a%128), ach, head, 65]; cols 0-63 = V head slice,
            # col 64 = ones (supplies softmax row-sums during AV).
            vaug = kvpool.tile([128, 4, H, DH + 1], BF16, name="vaug")
            nc.vector.memset(vaug, 1.0)

            # fixed parity tiles for the 1/sums chain; rows other than 0/64
            # hold 1.0 forever so the reciprocal/cast stay finite
            sums4s, rcpbfs = [], []
            rcp4 = cpool.tile([65, 2, 512], F32, name="rcp4")
            for eo in range(2):
                s4 = cpool.tile([65, 2, 512], F32, name=f"sums4_{eo}")
                nc.vector.memset(s4, 1.0)
                sums4s.append(s4)
                rb = cpool.tile([65, 2, 512], BF16, name=f"rcpbf_{eo}")
                rcpbfs.append(rb)

            qtz = []
            for rc in range(2):
                qt_z0 = qtpool.tile([128, 8, 512], BF16, tag=f"qt0_{rc}",
                                    name=f"qt_z0_{rc}", bufs=1)
                qt_z1 = qtpool.tile([128, 8, 512], BF16, tag=f"qt1_{rc}",
                                    name=f"qt_z1_{rc}", bufs=1)
                nc.vector.memset(qt_z0[64:128, :, :], 0.0)
                nc.vector.memset(qt_z1[0:64, :, :], 0.0)
                qtz.append((qt_z0, qt_z1))

            kt_sb = kvpool.tile([128, 8, A], BF16, name="kt_sb")

            # ---- K^T projection: kt[c, a] = (anchors @ Wk)^T ----
            for ct in range(8):
                pk = psum.tile([128, A], F32, tag="work", name="pk")
                for dt in range(8):
                    nc.tensor.matmul(
                        pk, wk_sb[:, dt, ct * 128:(ct + 1) * 128],
                        at_sb[:, dt, :], start=(dt == 0), stop=(dt == 7))
                nc.vector.tensor_scalar_add(
                    kt_sb[:, ct, :], pk, bk_sb[:, ct:ct + 1])

            # ---- Q^T projection for one head-pair group, written into two
            # zero-padded slabs (z0: odd-head partitions zeroed, z1: even)
            # so score matmuls contract over the full 128 partitions. The pq
            # PSUM tile shares the score tiles' "s" tag/rotation. ----
            def qproj(ct):
                for rc in range(2):
                    wsel = wlo_sb if rc == 0 else whi_sb
                    bsel = blo_sb if rc == 0 else bhi_sb
                    qt_z0, qt_z1 = qtz[rc]
                    pq = psum.tile([128, 512], F32, tag="s", name="pq",
                                   bufs=2)
                    for dt in range(8):
                        nc.tensor.matmul(
                            pq, wsel[:, dt, ct * 128:(ct + 1) * 128],
                            xt_sb[:, dt, rc * 512:(rc + 1) * 512],
                            start=(dt == 0), stop=(dt == 7))
                    nc.vector.tensor_scalar_add(
                        qt_z0[0:64, ct, :], pq[0:64, :], bsel[0:64, ct:ct + 1])
                    nc.vector.tensor_scalar_add(
                        qt_z1[64:128, ct, :], pq[64:128, :],
                        bsel[64:128, ct:ct + 1])

            qproj(0)
            qproj(1)

            # ---- attention, software-pipelined over the 8 head-pair
            # groups (ct): scores+exp run one group ahead of AV, two ahead
            # of the normalization; group ct+1's Q projection is emitted at
            # the tail of stage ct so the PE never crosses a phase barrier.
            wkstack.close()
            attnT = attnpool.tile([128, 8, RPC], BF16, name="attnT")

            def vchunk(ach, ch):
                pv = psum.tile([128, 512], F32, tag="work", name="pv")
                for dt in range(8):
                    nc.tensor.matmul(
                        pv, at_sb[:, dt, ach * 128:(ach + 1) * 128],
                        wv_sb[:, dt, ch * 512:(ch + 1) * 512],
                        start=(dt == 0), stop=(dt == 7))
                pv_v = pv.rearrange("p (hd d) -> p hd d", d=DH)
                bv_v = bv_bc.rearrange(
                    "p (chd hd d) -> p chd hd d", chd=2, d=DH)[:, ch]
                nc.vector.tensor_add(
                    vaug[:, ach, ch * 8:(ch + 1) * 8, 0:DH], pv_v, bv_v)

            def score_pair(ct, st, par, rc, half):
                # two score matmuls + one exp for heads {2ct, 2ct+1}
                if half == 0:
                    st["pts"][(par, rc)] = ptpool.tile(
                        [128, 4, 512], BF16, tag="pt", name="pt")
                pt = st["pts"][(par, rc)]
                qt_sb = qtz[rc][par]
                s2 = psum.tile([128, 2, 512], F32, tag="s", name="s2", bufs=2)
                for k in range(2):
                    ach = 2 * half + k
                    nc.tensor.matmul(
                        s2[:, k, :], kt_sb[:, ct, ach * 128:(ach + 1) * 128],
                        qt_sb[:, ct, :], start=True, stop=True)
                nc.scalar.activation(
                    out=pt[:, 2 * half:2 * half + 2, :], in_=s2,
                    func=Exp, scale=SCALE)

            def av_mm(ct, par, st, rc):
                # AV accumulation chain for one row-chunk
                h = 2 * ct + par
                if rc == 0:
                    st[f"pav{par}"] = psum.tile([128, 2, 512], F32,
                                                tag="work", name="pav",
                                                bufs=2)
                pav = st[f"pav{par}"]
                pt = st["pts"][(par, rc)]
                for ach in range(4):
                    nc.tensor.matmul(
                        pav[0:DH + 1, rc, :], vaug[:, ach, h, :],
                        pt[:, ach, :], start=(ach == 0), stop=(ach == 3))
                if rc == 1:
                    row = par * 64
                    if par == 0:
                        st["praw2"] = tmppool.tile([128, 2, 512], BF16,
                                                   tag="praw", name="praw2")
                        nc.scalar.copy(st["praw2"][row:row + DH, :, :],
                                       pav[0:DH, :, :])
                    else:
                        nc.vector.tensor_copy(st["praw2"][row:row + DH, :, :],
                                              pav[0:DH, :, :])
                    nc.vector.tensor_copy(sums4s[ct % 2][row:row + 1, :, :],
                                          pav[DH:DH + 1, :, :])

            def stage_recip(ct, st):
                nc.vector.reciprocal_approx_fast(rcp4, sums4s[ct % 2])
                nc.vector.tensor_copy(rcpbfs[ct % 2], rcp4)

            def stage_norm(ct, st):
                pav1 = st["pav1"]
                for rcn in range(2):
                    nc.tensor.matmul(
                        pav1[:, rcn, :], sel_sb, rcpbfs[ct % 2][:, rcn, :],
                        start=True, stop=True)
                dst = attnT[:, ct, :].rearrange("p (b r) -> p b r", b=2)
                nc.vector.tensor_mul(dst, st["praw2"], pav1)

            # O-proj partials for the first tiles are emitted inside the
            # pipeline drain so the PE has work while the last groups'
            # normalization chains run; the ones at i==8 borrow the then-idle
            # score-tag PSUM slots.
            pouts_head = {}

            def oproj_partial(rti, nh, tag, upto):
                pout = psum.tile([128, 512], F32, tag=tag, name="pout")
                for ct2 in range(upto):
                    nc.tensor.matmul(
                        pout, attnT[:, ct2, rti * 128:(rti + 1) * 128],
                        hold["wo_sb"][:, ct2, nh * 512:(nh + 1) * 512],
                        start=(ct2 == 0), stop=False)
                pouts_head[(rti, nh)] = (pout, upto)

            # Main pipeline: per stage i, the 8 exp-gated score pairs of
            # group i are interleaved with independent PE fill (AV of group
            # i-1, Q proj of group i+2, V projection at stage 0) so the PE
            # never waits on the scalar engine's exp stream.
            sts = {}
            for i in range(10):
                if i == 1:
                    projstack.close()
                    wopool = attnstack.enter_context(
                        tc.tile_pool(name="wopool", bufs=1))
                    hold["wo_sb"] = slab_split(wopool, wo, D, "wo_sb", nc.sync)
                fills = []
                if i == 0:
                    for ach in range(4):
                        for ch in range(2):
                            fills.append(lambda ach=ach, ch=ch: vchunk(ach, ch))
                if 1 <= i <= 8:
                    st_p = sts[i - 1]
                    fills.append(lambda st_p=st_p, i=i: av_mm(i - 2 + 1, 0, st_p, 0))
                    fills.append(lambda st_p=st_p, i=i: av_mm(i - 2 + 1, 0, st_p, 1))
                if i <= 5:
                    fills.append(lambda i=i: qproj(i + 2))
                if i < 8:
                    st = sts[i] = {"pts": {}}
                    pairs = [(par, rc, half) for par in range(2)
                             for rc in range(2) for half in range(2)]
                    fi = 0
                    for pi, (par, rc, half) in enumerate(pairs):
                        score_pair(i, st, par, rc, half)
                        if pi >= 1 and fi < len(fills) and pi < 6:
                            fills[fi](); fi += 1
                    while fi < len(fills):
                        fills[fi](); fi += 1
                else:
                    fi = 0
                    while fi < len(fills):
                        fills[fi](); fi += 1
                if i == 9:
                    oproj_partial(1, 0, "work", 7)
                if 2 <= i <= 9:
                    stage_recip(i - 2, sts[i - 2])
                    stage_norm(i - 2, sts[i - 2])
                if 1 <= i <= 8:
                    st_p = sts[i - 1]
                    av_mm(i - 1, 1, st_p, 0)
                    av_mm(i - 1, 1, st_p, 1)
                if i == 8:
                    oproj_partial(0, 0, "s", 7)
                    oproj_partial(0, 1, "s", 7)

            # ---- output projection ----
            oproj_partial(1, 1, "work", 0)
            for rti in range(8):
                for nh in range(2):
                    if (rti, nh) in pouts_head:
                        pout, upto = pouts_head[(rti, nh)]
                        for ct2 in range(upto, 8):
                            nc.tensor.matmul(
                                pout, attnT[:, ct2, rti * 128:(rti + 1) * 128],
                                hold["wo_sb"][:, ct2, nh * 512:(nh + 1) * 512],
                                start=(ct2 == 0), stop=(ct2 == 7))
                    else:
                        pout = psum.tile([128, 512], F32, tag="work",
                                         name="pout")
                        for ct2 in range(8):
                            nc.tensor.matmul(
                                pout, attnT[:, ct2, rti * 128:(rti + 1) * 128],
                                hold["wo_sb"][:, ct2, nh * 512:(nh + 1) * 512],
                                start=(ct2 == 0), stop=(ct2 == 7))
                    out_t = outpool.tile([128, 512], F32, tag="out",
                                         name="out_t")
                    nc.vector.tensor_add(out_t, pout,
                                         bo_bc[:, nh * 512:(nh + 1) * 512])
                    nc.sync.dma_start(
                        out=out.ap()[rti * 128:(rti + 1) * 128,
                                     nh * 512:(nh + 1) * 512],
                        in_=out_t)
            attnstack.close()

    nc.compile()
    return nc


def _swz(a):
    """[1024, cols] -> [128, 8, cols] with row r -> (r % 128, r // 128)."""
    return np.ascontiguousarray(
        a.reshape(8, 128, -1).transpose(1, 0, 2))


def _make_in_maps(x, Wq, bq, Wk, bk, Wv, bv, Wqt, bqt, Wo, bo):
    x = np.asarray(x, dtype=np.float32)
    bf = ml_dtypes.bfloat16

    wq_b = np.ascontiguousarray(np.asarray(Wq, np.float32).astype(bf))
    wqt_b = np.ascontiguousarray(np.asarray(Wqt, np.float32).astype(bf))
    wk_b = np.ascontiguousarray(np.asarray(Wk, np.float32).astype(bf))
    wv_b = np.ascontiguousarray(np.asarray(Wv, np.float32).astype(bf))
    wo_b = np.ascontiguousarray(np.asarray(Wo, np.float32).astype(bf))
    colmaj = lambda v: np.ascontiguousarray(
        np.asarray(v, np.float32).reshape(8, 128).T)
    bq, bqt, bk = map(colmaj, (bq, bqt, bk))
    bv = np.ascontiguousarray(
        np.broadcast_to(np.asarray(bv, np.float32).astype(bf), (128, D)))
    bo = np.ascontiguousarray(
        np.broadcast_to(np.asarray(bo, np.float32).astype(bf), (128, D)))

    wq_sw, wqt_sw = _swz(wq_b), _swz(wqt_b)
    wk_sw, wv_sw, wo_sw = _swz(wk_b), _swz(wv_b), _swz(wo_b)
    at_sw = [_swz(x[b, :A, :].T.astype(bf)) for b in range(B)]
    in_maps = []
    for c in range(NCORES):
        b, q = divmod(c, 4)
        rows = x[b, q * RPC:(q + 1) * RPC, :]
        in_maps.append({
            "xt": _swz(rows.T.astype(bf)),
            "at": at_sw[b],
            "wlo": wq_sw if q == 0 else wqt_sw,
            "whi": wqt_sw,
            "wk": wk_sw, "wv": wv_sw, "wo": wo_sw,
            "blo": bq if q == 0 else bqt, "bhi": bqt,
            "bk": bk, "bv": bv, "bo": bo,
        })
    return in_maps


def kernel(x, Wq, bq, Wk, bk, Wv, bv, Wqt, bqt, Wo, bo, num_anchor_tokens):
    assert int(num_anchor_tokens) == A
    if "nc" not in _CACHE:
        _CACHE["nc"] = _build()
    nc = _CACHE["nc"]

    in_maps = _make_in_maps(x, Wq, bq, Wk, bk, Wv, bv, Wqt, bqt, Wo, bo)
    res = bass_utils.run_bass_kernel_spmd(
        nc, in_maps, core_ids=list(range(NCORES)))
    out = np.empty((B, S, D), np.float32)
    for c in range(NCORES):
        b, q = divmod(c, 4)
        out[b, q * RPC:(q + 1) * RPC, :] = res.results[c]["out"]
    return out
